# revision 15
# baseline (speedup 1.0000x reference)
"""Trainium2 Bass kernel for nn_BimodalCrossAttentionBlock.

Math: seq-len-1 multihead cross attention => softmax over a single key is
identically 1, so MHA(x_q, x_kv) collapses to out_proj(v_proj(x_kv)) and the
two projections fold into one matrix Wc = out_w @ in_w[2D:] (Q/K projections
and num_heads are dead).  The block then is:
  graph_res = LN(graph + seq @ Wc_s2g.T + bc_s2g)     (gn1)
  seq_res   = LN(seq + graph @ Wc_g2s.T + bc_g2s)     (sn1)
  seq_out   = LN(seq_res + FFN_seq(seq_res))          (sn2)
  graph_out = LN(graph_res + FFN_gr(graph_res))       (gn2)

Sharding: pure data parallel over the batch dim (4096 rows/core x 8 cores);
weights replicated, no collectives.  Matmuls run in fp16 (fp32 PSUM accum),
skip paths / LayerNorm in fp32.  Host supplies the inputs in f16 in BOTH
batch-major and feature-major (pre-transposed) layouts, so phase A needs no
on-device input transposes.  Three on-device phases:
  A: attention + LN1 for both modalities (activation-stationary matmuls,
     batch-major outputs; LN rsqrt via Newton iteration on DVE); emits res in
     batch-major f16 (skip path) and feature-major f16 (FFN operand).
  B: seq FFN + LN2 (w1 prefetched during phase A; w2 streamed in ht-major
     chunks so the first matmuls start immediately)
  C: graph FFN + LN2
"""
import numpy as np

import concourse.bass as bass
import concourse.bacc as bacc
import concourse.tile as tile
import concourse.mybir as mybir
from concourse.bass_utils import run_bass_kernel_spmd
from concourse.masks import make_identity

F16 = mybir.dt.float16
F32 = mybir.dt.float32
U32 = mybir.dt.uint32
AF = mybir.ActivationFunctionType
ALU = mybir.AluOpType

N_CORES = 8
B_FULL = 32768
D = 1024
HID = 4096
R = B_FULL // N_CORES
EPS = 1e-5
MAGIC = 0x5F3759DF

_cache = {}


def _ln_tail(nc, work, magic, x2, out_tile, lng_bc, lnb_bc):
    """LayerNorm of x2 [128, D] f32 -> out_tile; stats + rsqrt all on DVE."""
    stats = work.tile([128, 2, 6], F32, tag="lnstats")
    mv = work.tile([128, 2], F32, tag="lnmv")
    nc.vector.bn_stats(out=stats[:, 0, :], in_=x2[:, 0:512])
    nc.vector.bn_stats(out=stats[:, 1, :], in_=x2[:, 512:1024])
    nc.vector.bn_aggr(out=mv, in_=stats)
    v = work.tile([128, 1], F32, tag="lnv")
    nc.vector.tensor_scalar(out=v, in0=mv[:, 1:2], scalar1=EPS, scalar2=None,
                            op0=ALU.add)
    y = work.tile([128, 1], F32, tag="lny")
    t = work.tile([128, 1], F32, tag="lnt")
    nc.vector.tensor_scalar(out=y.bitcast(U32), in0=v.bitcast(U32), scalar1=1,
                            scalar2=None, op0=ALU.logical_shift_right)
    nc.vector.tensor_tensor(out=y.bitcast(U32), in0=magic, in1=y.bitcast(U32),
                            op=ALU.subtract)
    for _ in range(3):
        nc.vector.tensor_mul(out=t, in0=y, in1=y)
        nc.vector.tensor_mul(out=t, in0=t, in1=v)
        nc.vector.tensor_scalar(out=t, in0=t, scalar1=-0.5, scalar2=1.5,
                                op0=ALU.mult, op1=ALU.add)
        nc.vector.tensor_mul(out=y, in0=y, in1=t)
    if lng_bc is None and lnb_bc is None:
        nc.vector.tensor_scalar(out=out_tile, in0=x2, scalar1=mv[:, 0:1],
                                scalar2=y, op0=ALU.subtract, op1=ALU.mult)
    else:
        tmp = work.tile([128, 1024], F32, tag="lntmp")
        nc.vector.tensor_scalar(out=tmp, in0=x2, scalar1=mv[:, 0:1],
                                scalar2=y, op0=ALU.subtract, op1=ALU.mult)
        if lng_bc is not None:
            nc.vector.tensor_mul(out=tmp, in0=tmp, in1=lng_bc)
        if lnb_bc is not None:
            nc.vector.tensor_add(out=out_tile, in0=tmp, in1=lnb_bc)
        else:
            nc.vector.tensor_copy(out=out_tile, in_=tmp)


def _ln_dual_fast(nc, work, magic, x_s, x_g, res_s, res_g):
    """LayerNorm (no affine) of two [128, D] tiles; stats on DVE, the final
    normalize on the scalar engine (out = Identity(x * rstd - mu*rstd))."""
    stats = work.tile([128, 2, 2, 6], F32, tag="lnstats2")
    mv = work.tile([128, 2, 2], F32, tag="lnmv2")
    for i, x in enumerate((x_s, x_g)):
        nc.vector.bn_stats(out=stats[:, i, 0, :], in_=x[:, 0:512])
        nc.vector.bn_stats(out=stats[:, i, 1, :], in_=x[:, 512:1024])
        nc.vector.bn_aggr(out=mv[:, i, :], in_=stats[:, i, :, :])
    v = work.tile([128, 2], F32, tag="lnv2")
    nc.vector.tensor_scalar(out=v, in0=mv[:, :, 1], scalar1=EPS, scalar2=None,
                            op0=ALU.add)
    y = work.tile([128, 2], F32, tag="lny2")
    t = work.tile([128, 2], F32, tag="lnt2")
    nc.vector.tensor_scalar(out=y.bitcast(U32), in0=v.bitcast(U32), scalar1=1,
                            scalar2=None, op0=ALU.logical_shift_right)
    nc.vector.tensor_tensor(out=y.bitcast(U32), in0=magic, in1=y.bitcast(U32),
                            op=ALU.subtract)
    for _ in range(3):
        nc.vector.tensor_mul(out=t, in0=y, in1=y)
        nc.vector.tensor_mul(out=t, in0=t, in1=v)
        nc.vector.tensor_scalar(out=t, in0=t, scalar1=-0.5, scalar2=1.5,
                                op0=ALU.mult, op1=ALU.add)
        nc.vector.tensor_mul(out=y, in0=y, in1=t)
    nmy = work.tile([128, 2], F32, tag="lnnmy2")
    nc.vector.tensor_tensor(out=nmy, in0=mv[:, :, 0], in1=y, op=ALU.mult)
    nc.vector.tensor_scalar(out=nmy, in0=nmy, scalar1=-1.0, scalar2=None,
                            op0=ALU.mult)
    for i, (x, res) in enumerate(((x_s, res_s), (x_g, res_g))):
        nc.scalar.activation(out=res, in_=x, func=AF.Identity,
                             scale=y[:, i:i + 1], bias=nmy[:, i:i + 1])


def _bcast_param(nc, pool, dram_ap, n, tag):
    t = pool.tile([128, n], F32, tag=tag)
    src = bass.AP(tensor=dram_ap.tensor, offset=dram_ap.offset,
                  ap=[[0, 128]] + dram_ap.ap)
    nc.gpsimd.dma_start(out=t, in_=src)
    return t


def _build(flags):
    fl = lambda k: bool(flags.get(k, False))
    nc = bacc.Bacc("TRN2", target_bir_lowering=False, debug=False,
                   num_devices=N_CORES)

    seq16 = nc.declare_dram_parameter("seq16", [R, D], F16, isOutput=False)
    graph16 = nc.declare_dram_parameter("graph16", [R, D], F16, isOutput=False)
    seqT = nc.declare_dram_parameter("seqT", [128, 8, R], F16, isOutput=False)
    graphT = nc.declare_dram_parameter("graphT", [128, 8, R], F16, isOutput=False)
    wcs = nc.declare_dram_parameter("wcs", [128, 8, D], F16, isOutput=False)
    wcg = nc.declare_dram_parameter("wcg", [128, 8, D], F16, isOutput=False)
    w1s = nc.declare_dram_parameter("w1s", [128, 8, HID], F16, isOutput=False)
    w2s = nc.declare_dram_parameter("w2s", [128, 32, D], F16, isOutput=False)
    w1g = nc.declare_dram_parameter("w1g", [128, 8, HID], F16, isOutput=False)
    w2g = nc.declare_dram_parameter("w2g", [128, 32, D], F16, isOutput=False)
    opt = {}
    for nm, shape, dt in [("bcs", [1, D], F16), ("bcg", [1, D], F16),
                          ("b1s", [128, 32], F32), ("b1g", [128, 32], F32),
                          ("b2s", [1, D], F16), ("b2g", [1, D], F16),
                          ("sn1_g", [D], F32), ("sn1_b", [D], F32),
                          ("sn2_g", [D], F32), ("sn2_b", [D], F32),
                          ("gn1_g", [D], F32), ("gn1_b", [D], F32),
                          ("gn2_g", [D], F32), ("gn2_b", [D], F32)]:
        if fl(nm):
            opt[nm] = nc.declare_dram_parameter(nm, shape, dt, isOutput=False)
    seq_out = nc.declare_dram_parameter("seq_out", [R, D], F32, isOutput=True)
    graph_out = nc.declare_dram_parameter("graph_out", [R, D], F32, isOutput=True)

    NT = R // 128
    NB = R // 256

    with tile.TileContext(nc) as tc:
        with tc.tile_pool(name="dram", bufs=1, space="DRAM") as dram_pool, \
             tc.tile_pool(name="wpre", bufs=1) as wpre:
            sB_s = dram_pool.tile([R, D], F16)
            sB_g = dram_pool.tile([R, D], F16)
            sT_s = dram_pool.tile([128, 8, R], F16)
            sT_g = dram_pool.tile([128, 8, R], F16)

            # One w1 buffer shared by phases B and C (re-filled for C); the
            # seq-FFN w1 prefetches during phase A in ht-major chunks.
            w1_sb = wpre.tile([128, 8, HID], F16)
            for hs in range(8):
                nc.sync.dma_start(out=w1_sb[:, :, hs * 512:(hs + 1) * 512],
                                  in_=w1s[:, :, hs * 512:(hs + 1) * 512])

            # ---------------- Phase A: attention + LN1 ----------------
            with tc.tile_pool(name="singlesA", bufs=1) as singles, \
                 tc.tile_pool(name="workA", bufs=4) as work, \
                 tc.tile_pool(name="pstpA", bufs=3, space="PSUM") as pstp, \
                 tc.tile_pool(name="psmmA", bufs=5, space="PSUM") as psmm:

                wcs_sb = singles.tile([128, 8, D], F16)
                wcg_sb = singles.tile([128, 8, D], F16)
                for kt in range(8):
                    nc.sync.dma_start(out=wcs_sb[:, kt, :], in_=wcs[:, kt, :])
                    nc.sync.dma_start(out=wcg_sb[:, kt, :], in_=wcg[:, kt, :])
                ident16 = singles.tile([128, 128], F16)
                make_identity(nc, ident16)
                magic = singles.tile([128, 2], U32)
                nc.vector.memset(magic, MAGIC)
                ones16 = None
                opt_sb = {}
                if fl("bcs") or fl("bcg"):
                    ones16 = singles.tile([1, 128], F16)
                    nc.vector.memset(ones16, 1.0)
                    for nm in ("bcs", "bcg"):
                        if nm in opt:
                            opt_sb[nm] = singles.tile([1, D], F16, name=f"sb_{nm}")
                            nc.sync.dma_start(out=opt_sb[nm], in_=opt[nm][:, :])
                ln_bcs = {}
                for nm in ("sn1_g", "sn1_b", "gn1_g", "gn1_b"):
                    if nm in opt:
                        ln_bcs[nm] = _bcast_param(nc, singles, opt[nm].ap(), D, nm)

                for t in range(NT):
                    row = t * 128
                    S16 = work.tile([128, D], F16, tag="S16")
                    nc.sync.dma_start(out=S16, in_=seq16[row:row + 128, :])
                    G16 = work.tile([128, D], F16, tag="G16")
                    nc.sync.dma_start(out=G16, in_=graph16[row:row + 128, :])
                    ST = work.tile([128, 8, 128], F16, tag="ST")
                    nc.sync.dma_start(out=ST, in_=seqT[:, :, row:row + 128])
                    GT = work.tile([128, 8, 128], F16, tag="GT")
                    nc.sync.dma_start(out=GT, in_=graphT[:, :, row:row + 128])

                    x_s = work.tile([128, D], F16, tag="xs")
                    x_g = work.tile([128, D], F16, tag="xg")
                    # interleave the four attention psum tiles; evacuate each
                    # with its residual add as soon as its group completes so
                    # the next tile's matmuls aren't starved of PSUM.
                    for half in range(2):
                        nsl = slice(half * 512, (half + 1) * 512)
                        ga = psmm.tile([128, 512], F32, tag="attnps",
                                       name=f"ga{t}_{half}")
                        for kt in range(8):
                            nc.tensor.matmul(ga, lhsT=ST[:, kt, :],
                                             rhs=wcs_sb[:, kt, nsl],
                                             start=(kt == 0),
                                             stop=(kt == 7 and not fl("bcs")))
                        if fl("bcs"):
                            nc.tensor.matmul(ga, lhsT=ones16,
                                             rhs=opt_sb["bcs"][:, nsl],
                                             start=False, stop=True)
                        nc.vector.tensor_add(out=x_g[:, nsl], in0=G16[:, nsl],
                                             in1=ga)
                        sa = psmm.tile([128, 512], F32, tag="attnps",
                                       name=f"sa{t}_{half}")
                        for kt in range(8):
                            nc.tensor.matmul(sa, lhsT=GT[:, kt, :],
                                             rhs=wcg_sb[:, kt, nsl],
                                             start=(kt == 0),
                                             stop=(kt == 7 and not fl("bcg")))
                        if fl("bcg"):
                            nc.tensor.matmul(sa, lhsT=ones16,
                                             rhs=opt_sb["bcg"][:, nsl],
                                             start=False, stop=True)
                        nc.vector.tensor_add(out=x_s[:, nsl], in0=S16[:, nsl],
                                             in1=sa)

                    res16_s = work.tile([128, D], F16, tag="res16s")
                    res16_g = work.tile([128, D], F16, tag="res16g")
                    if ln_bcs:
                        for x, res16, g_nm, b_nm in (
                            (x_s, res16_s, "sn1_g", "sn1_b"),
                            (x_g, res16_g, "gn1_g", "gn1_b"),
                        ):
                            _ln_tail(nc, work, magic[:, 0:1], x, res16,
                                     ln_bcs.get(g_nm), ln_bcs.get(b_nm))
                    else:
                        _ln_dual_fast(nc, work, magic, x_s, x_g,
                                      res16_s, res16_g)

                    for which, res16, sB_d, sT_d in (
                        ("s", res16_s, sB_s, sT_s),
                        ("g", res16_g, sB_g, sT_g),
                    ):
                        rTt = work.tile([128, 8, 128], F16, tag=f"rTt{which}")
                        for grp in range(2):
                            tpr = pstp.tile([128, 512], F16, tag="tp",
                                            name=f"tpr{which}{t}_{grp}")
                            for j in range(4):
                                kt = grp * 4 + j
                                nc.tensor.transpose(tpr[:, j * 128:(j + 1) * 128],
                                                    res16[:, kt * 128:(kt + 1) * 128],
                                                    ident16)
                            nc.vector.tensor_copy(
                                out=rTt[:, grp * 4:(grp + 1) * 4, :].rearrange("p a b -> p (a b)"),
                                in_=tpr)
                        nc.sync.dma_start(out=sT_d[:, :, row:row + 128], in_=rTt)
                        nc.sync.dma_start(out=sB_d[row:row + 128, :], in_=res16)

            # ---------------- Phases B/C: FFN + LN2 ----------------
            for ph, (w1_in, w2_in, sB_d, sT_d, out_d, b1_nm, b2_nm, g_nm, b_nm) in enumerate((
                (w1s, w2s, sB_s, sT_s, seq_out, "b1s", "b2s", "sn2_g", "sn2_b"),
                (w1g, w2g, sB_g, sT_g, graph_out, "b1g", "b2g", "gn2_g", "gn2_b"),
            )):
                with tc.tile_pool(name=f"singles{ph}", bufs=1) as singles, \
                     tc.tile_pool(name=f"work{ph}", bufs=3) as work, \
                     tc.tile_pool(name=f"hg{ph}", bufs=12) as hgpool, \
                     tc.tile_pool(name=f"psh{ph}", bufs=3, space="PSUM") as psh, \
                     tc.tile_pool(name=f"pso{ph}", bufs=5, space="PSUM") as pso:

                    # stage the first block's activations ahead of the bulk
                    # weight DMA so the phase's first matmuls aren't queued
                    # behind 8+ MB of weight traffic.
                    rT0 = work.tile([128, 8, 256], F16, tag="rT")
                    nc.sync.dma_start(out=rT0, in_=sT_d[:, :, 0:256])
                    rB0 = work.tile([128, 2, D], F16, tag="rB")
                    nc.sync.dma_start(
                        out=rB0,
                        in_=sB_d[0:256, :].rearrange("(s p) n -> p s n", p=128))
                    if ph != 0:
                        # refill the shared w1 buffer for the graph FFN;
                        # dependency tracking delays each chunk's DMA until
                        # phase B's reads of that region have completed.
                        for hs in range(8):
                            nc.sync.dma_start(
                                out=w1_sb[:, :, hs * 512:(hs + 1) * 512],
                                in_=w1_in[:, :, hs * 512:(hs + 1) * 512])
                    w2_sb = singles.tile([128, 32, D], F16)
                    for ktg in range(8):
                        nc.sync.dma_start(out=w2_sb[:, ktg * 4:(ktg + 1) * 4, :],
                                          in_=w2_in[:, ktg * 4:(ktg + 1) * 4, :])
                    magic = singles.tile([128, 1], U32)
                    nc.vector.memset(magic, MAGIC)
                    b1_sb = None
                    if b1_nm in opt:
                        b1_sb = singles.tile([128, 32], F32)
                        nc.sync.dma_start(out=b1_sb, in_=opt[b1_nm][:, :])
                    ones16 = None
                    b2_sb = None
                    if b2_nm in opt:
                        ones16 = singles.tile([1, 128], F16)
                        nc.vector.memset(ones16, 1.0)
                        b2_sb = singles.tile([1, D], F16)
                        nc.sync.dma_start(out=b2_sb, in_=opt[b2_nm][:, :])
                    ln_g_bc = (_bcast_param(nc, singles, opt[g_nm].ap(), D, g_nm)
                               if g_nm in opt else None)
                    ln_b_bc = (_bcast_param(nc, singles, opt[b_nm].ap(), D, b_nm)
                               if b_nm in opt else None)

                    for blk in range(NB):
                        brow = blk * 256
                        if blk == 0:
                            rT, rB = rT0, rB0
                        else:
                            rT = work.tile([128, 8, 256], F16, tag="rT")
                            nc.sync.dma_start(out=rT, in_=sT_d[:, :, brow:brow + 256])
                            rB = work.tile([128, 2, D], F16, tag="rB")
                            nc.sync.dma_start(
                                out=rB,
                                in_=sB_d[brow:brow + 256, :].rearrange("(s p) n -> p s n", p=128))
                        ops = [pso.tile([128, 512], F32, tag="ops", name=f"ops{blk}_{_h}")
                               for _h in range(4)]
                        for ht in range(32):
                            hps = psh.tile([128, 256], F32, tag="hps")
                            for kt in range(8):
                                nc.tensor.matmul(hps,
                                                 lhsT=w1_sb[:, kt, ht * 128:(ht + 1) * 128],
                                                 rhs=rT[:, kt, :],
                                                 start=(kt == 0), stop=(kt == 7))
                            # two half-gelus into separate tiles: the bs=0
                            # matmul's weight load can start as soon as the
                            # first half lands instead of waiting for the
                            # full [128,256] activation to finish.
                            hgs = [hgpool.tile([128, 128], F16, tag="hg",
                                               name=f"hg{blk}_{ht}_{_b}")
                                   for _b in range(2)]
                            for bs in range(2):
                                if b1_sb is not None:
                                    nc.scalar.activation(
                                        out=hgs[bs], in_=hps[:, bs * 128:(bs + 1) * 128],
                                        func=AF.Gelu, bias=b1_sb[:, ht:ht + 1],
                                        scale=1.0, alpha=0.0)
                                else:
                                    nc.scalar.activation(
                                        out=hgs[bs], in_=hps[:, bs * 128:(bs + 1) * 128],
                                        func=AF.Gelu)
                            for bs in range(2):
                                for nh in range(2):
                                    nc.tensor.matmul(
                                        ops[bs * 2 + nh],
                                        lhsT=hgs[bs],
                                        rhs=w2_sb[:, ht, nh * 512:(nh + 1) * 512],
                                        start=(ht == 0),
                                        stop=(ht == 31 and b2_sb is None))
                        if b2_sb is not None:
                            for bs in range(2):
                                for nh in range(2):
                                    nc.tensor.matmul(ops[bs * 2 + nh], lhsT=ones16,
                                                     rhs=b2_sb[:, nh * 512:(nh + 1) * 512],
                                                     start=False, stop=True)
                        for bs in range(2):
                            x2 = work.tile([128, D], F32, tag="x2")
                            nc.vector.tensor_add(out=x2[:, 0:512], in0=rB[:, bs, 0:512],
                                                 in1=ops[bs * 2 + 0])
                            nc.vector.tensor_add(out=x2[:, 512:1024],
                                                 in0=rB[:, bs, 512:1024],
                                                 in1=ops[bs * 2 + 1])
                            ot = work.tile([128, D], F32, tag="ot")
                            _ln_tail(nc, work, magic, x2, ot, ln_g_bc, ln_b_bc)
                            nc.sync.dma_start(
                                out=out_d[brow + bs * 128:brow + bs * 128 + 128, :],
                                in_=ot)

    nc.compile()
    return nc


def _host_prep(inputs):
    f = lambda k: np.asarray(inputs[k])
    flags = {}

    def fold(pfx):
        in_w = f(f"{pfx}_in_w").astype(np.float64)
        in_b = f(f"{pfx}_in_b").astype(np.float64)
        out_w = f(f"{pfx}_out_w").astype(np.float64)
        out_b = f(f"{pfx}_out_b").astype(np.float64)
        Wc = out_w @ in_w[2 * D:]
        bc = in_b[2 * D:] @ out_w.T + out_b
        return Wc, bc

    Wcs, bcs = fold("s2g")
    Wcg, bcg = fold("g2s")

    def rhs_tiles(W, kt):  # W [n, d_in] -> [128, kt, n] f16 tiles of W.T
        return np.ascontiguousarray(
            W.T.reshape(kt, 128, -1).transpose(1, 0, 2)).astype(np.float16)

    wm = {
        "wcs": rhs_tiles(Wcs, 8), "wcg": rhs_tiles(Wcg, 8),
        "w1s": rhs_tiles(f("seq_w1"), 8), "w2s": rhs_tiles(f("seq_w2"), 32),
        "w1g": rhs_tiles(f("gr_w1"), 8), "w2g": rhs_tiles(f("gr_w2"), 32),
    }
    if np.any(bcs != 0):
        flags["bcs"] = True
        wm["bcs"] = bcs.astype(np.float16).reshape(1, D)
    if np.any(bcg != 0):
        flags["bcg"] = True
        wm["bcg"] = bcg.astype(np.float16).reshape(1, D)
    for nm, key in (("b1s", "seq_b1"), ("b1g", "gr_b1")):
        v = f(key)
        if np.any(v != 0):
            flags[nm] = True
            wm[nm] = np.ascontiguousarray(v.reshape(32, 128).T).astype(np.float32)
    for nm, key in (("b2s", "seq_b2"), ("b2g", "gr_b2")):
        v = f(key)
        if np.any(v != 0):
            flags[nm] = True
            wm[nm] = v.astype(np.float16).reshape(1, D)
    for nm in ("sn1", "sn2", "gn1", "gn2"):
        g = f(f"{nm}_g"); b = f(f"{nm}_b")
        if np.any(g != 1):
            flags[f"{nm}_g"] = True
            wm[f"{nm}_g"] = g.astype(np.float32)
        if np.any(b != 0):
            flags[f"{nm}_b"] = True
            wm[f"{nm}_b"] = b.astype(np.float32)

    seq = np.asarray(f("seq_emb"), dtype=np.float32)
    graph = np.asarray(f("graph_emb"), dtype=np.float32)
    seq16_full = seq.astype(np.float16)
    graph16_full = graph.astype(np.float16)
    in_maps = []
    for i in range(N_CORES):
        m = dict(wm)
        s = seq16_full[i * R:(i + 1) * R]
        g = graph16_full[i * R:(i + 1) * R]
        m["seq16"] = np.ascontiguousarray(s)
        m["graph16"] = np.ascontiguousarray(g)
        m["seqT"] = rhs_tiles(s, 8)
        m["graphT"] = rhs_tiles(g, 8)
        in_maps.append(m)
    return in_maps, flags


def kernel(**inputs):
    in_maps, flags = _host_prep(inputs)
    key = tuple(sorted(flags.items()))
    if key not in _cache:
        _cache[key] = _build(flags)
    nc = _cache[key]
    res = run_bass_kernel_spmd(nc, in_maps, core_ids=list(range(N_CORES)))
    seq_out = np.concatenate([res.results[i]["seq_out"] for i in range(N_CORES)], axis=0)
    graph_out = np.concatenate([res.results[i]["graph_out"] for i in range(N_CORES)], axis=0)
    return (seq_out, graph_out)


# revision 18
# speedup vs baseline: 1.1630x; 1.1630x over previous
"""Trainium2 Bass kernel for nn_BimodalCrossAttentionBlock.

Math: seq-len-1 multihead cross attention => softmax over a single key is
identically 1, so MHA(x_q, x_kv) collapses to out_proj(v_proj(x_kv)) and the
two projections fold into one matrix Wc = out_w @ in_w[2D:] (Q/K projections
and num_heads are dead).  The block then is:
  graph_res = LN(graph + seq @ Wc_s2g.T + bc_s2g)     (gn1)
  seq_res   = LN(seq + graph @ Wc_g2s.T + bc_g2s)     (sn1)
  seq_out   = LN(seq_res + FFN_seq(seq_res))          (sn2)
  graph_out = LN(graph_res + FFN_gr(graph_res))       (gn2)

Sharding: pure data parallel over the batch dim (4096 rows/core x 8 cores);
weights replicated, no collectives.  Matmuls run in fp16 (fp32 PSUM accum),
skip paths / LayerNorm in fp32.  Host supplies the inputs in f16 in BOTH
batch-major and feature-major (pre-transposed) layouts, so phase A needs no
on-device input transposes.  Three on-device phases:
  A: attention + LN1 for both modalities (activation-stationary matmuls,
     batch-major outputs; LN rsqrt via Newton iteration on DVE); emits res in
     batch-major f16 (skip path) and feature-major f16 (FFN operand).
  B: seq FFN + LN2 (w1 prefetched during phase A; w2 streamed in ht-major
     chunks so the first matmuls start immediately)
  C: graph FFN + LN2
"""
import numpy as np

import concourse.bass as bass
import concourse.bacc as bacc
import concourse.tile as tile
import concourse.mybir as mybir
from concourse.bass_utils import run_bass_kernel_spmd
from concourse.masks import make_identity

F16 = mybir.dt.float16
F32 = mybir.dt.float32
U32 = mybir.dt.uint32
AF = mybir.ActivationFunctionType
ALU = mybir.AluOpType

N_CORES = 8
B_FULL = 32768
D = 1024
HID = 4096
R = B_FULL // N_CORES
EPS = 1e-5
MAGIC = 0x5F3759DF

_cache = {}


def _ln_tail(nc, work, magic, x2, out_tile, lng_bc, lnb_bc):
    """LayerNorm of x2 [128, D] f32 -> out_tile; stats + rsqrt all on DVE."""
    stats = work.tile([128, 2, 6], F32, tag="lnstats")
    mv = work.tile([128, 2], F32, tag="lnmv")
    nc.vector.bn_stats(out=stats[:, 0, :], in_=x2[:, 0:512])
    nc.vector.bn_stats(out=stats[:, 1, :], in_=x2[:, 512:1024])
    nc.vector.bn_aggr(out=mv, in_=stats)
    v = work.tile([128, 1], F32, tag="lnv")
    nc.vector.tensor_scalar(out=v, in0=mv[:, 1:2], scalar1=EPS, scalar2=None,
                            op0=ALU.add)
    y = work.tile([128, 1], F32, tag="lny")
    t = work.tile([128, 1], F32, tag="lnt")
    nc.vector.tensor_scalar(out=y.bitcast(U32), in0=v.bitcast(U32), scalar1=1,
                            scalar2=None, op0=ALU.logical_shift_right)
    nc.vector.tensor_tensor(out=y.bitcast(U32), in0=magic, in1=y.bitcast(U32),
                            op=ALU.subtract)
    for _ in range(3):
        nc.vector.tensor_mul(out=t, in0=y, in1=y)
        nc.vector.tensor_mul(out=t, in0=t, in1=v)
        nc.vector.tensor_scalar(out=t, in0=t, scalar1=-0.5, scalar2=1.5,
                                op0=ALU.mult, op1=ALU.add)
        nc.vector.tensor_mul(out=y, in0=y, in1=t)
    if lng_bc is None and lnb_bc is None:
        nc.vector.tensor_scalar(out=out_tile, in0=x2, scalar1=mv[:, 0:1],
                                scalar2=y, op0=ALU.subtract, op1=ALU.mult)
    else:
        tmp = work.tile([128, 1024], F32, tag="lntmp")
        nc.vector.tensor_scalar(out=tmp, in0=x2, scalar1=mv[:, 0:1],
                                scalar2=y, op0=ALU.subtract, op1=ALU.mult)
        if lng_bc is not None:
            nc.vector.tensor_mul(out=tmp, in0=tmp, in1=lng_bc)
        if lnb_bc is not None:
            nc.vector.tensor_add(out=out_tile, in0=tmp, in1=lnb_bc)
        else:
            nc.vector.tensor_copy(out=out_tile, in_=tmp)


def _ln_dual_fast(nc, work, magic, x_s, x_g, res_s, res_g):
    """LayerNorm (no affine) of two [128, D] tiles; stats on DVE, the final
    normalize on the scalar engine (out = Identity(x * rstd - mu*rstd))."""
    stats = work.tile([128, 2, 2, 6], F32, tag="lnstats2")
    mv = work.tile([128, 2, 2], F32, tag="lnmv2")
    for i, x in enumerate((x_s, x_g)):
        nc.vector.bn_stats(out=stats[:, i, 0, :], in_=x[:, 0:512])
        nc.vector.bn_stats(out=stats[:, i, 1, :], in_=x[:, 512:1024])
        nc.vector.bn_aggr(out=mv[:, i, :], in_=stats[:, i, :, :])
    v = work.tile([128, 2], F32, tag="lnv2")
    nc.vector.tensor_scalar(out=v, in0=mv[:, :, 1], scalar1=EPS, scalar2=None,
                            op0=ALU.add)
    y = work.tile([128, 2], F32, tag="lny2")
    t = work.tile([128, 2], F32, tag="lnt2")
    nc.vector.tensor_scalar(out=y.bitcast(U32), in0=v.bitcast(U32), scalar1=1,
                            scalar2=None, op0=ALU.logical_shift_right)
    nc.vector.tensor_tensor(out=y.bitcast(U32), in0=magic, in1=y.bitcast(U32),
                            op=ALU.subtract)
    for _ in range(3):
        nc.vector.tensor_mul(out=t, in0=y, in1=y)
        nc.vector.tensor_mul(out=t, in0=t, in1=v)
        nc.vector.tensor_scalar(out=t, in0=t, scalar1=-0.5, scalar2=1.5,
                                op0=ALU.mult, op1=ALU.add)
        nc.vector.tensor_mul(out=y, in0=y, in1=t)
    nmy = work.tile([128, 2], F32, tag="lnnmy2")
    nc.vector.tensor_tensor(out=nmy, in0=mv[:, :, 0], in1=y, op=ALU.mult)
    nc.vector.tensor_scalar(out=nmy, in0=nmy, scalar1=-1.0, scalar2=None,
                            op0=ALU.mult)
    for i, (x, res) in enumerate(((x_s, res_s), (x_g, res_g))):
        nc.scalar.activation(out=res, in_=x, func=AF.Identity,
                             scale=y[:, i:i + 1], bias=nmy[:, i:i + 1])


def _bcast_param(nc, pool, dram_ap, n, tag):
    t = pool.tile([128, n], F32, tag=tag)
    src = bass.AP(tensor=dram_ap.tensor, offset=dram_ap.offset,
                  ap=[[0, 128]] + dram_ap.ap)
    nc.gpsimd.dma_start(out=t, in_=src)
    return t


def _build(flags):
    fl = lambda k: bool(flags.get(k, False))
    nc = bacc.Bacc("TRN2", target_bir_lowering=False, debug=False,
                   num_devices=N_CORES)

    seq16 = nc.declare_dram_parameter("seq16", [R, D], F16, isOutput=False)
    graph16 = nc.declare_dram_parameter("graph16", [R, D], F16, isOutput=False)
    seqT = nc.declare_dram_parameter("seqT", [128, 8, R], F16, isOutput=False)
    graphT = nc.declare_dram_parameter("graphT", [128, 8, R], F16, isOutput=False)
    wcs = nc.declare_dram_parameter("wcs", [128, 8, D], F16, isOutput=False)
    wcg = nc.declare_dram_parameter("wcg", [128, 8, D], F16, isOutput=False)
    w1s = nc.declare_dram_parameter("w1s", [128, 8, HID], F16, isOutput=False)
    w2s = nc.declare_dram_parameter("w2s", [128, 32, D], F16, isOutput=False)
    w1g = nc.declare_dram_parameter("w1g", [128, 8, HID], F16, isOutput=False)
    w2g = nc.declare_dram_parameter("w2g", [128, 32, D], F16, isOutput=False)
    opt = {}
    for nm, shape, dt in [("bcs", [1, D], F16), ("bcg", [1, D], F16),
                          ("b1s", [128, 32], F32), ("b1g", [128, 32], F32),
                          ("b2s", [1, D], F16), ("b2g", [1, D], F16),
                          ("sn1_g", [D], F32), ("sn1_b", [D], F32),
                          ("sn2_g", [D], F32), ("sn2_b", [D], F32),
                          ("gn1_g", [D], F32), ("gn1_b", [D], F32),
                          ("gn2_g", [D], F32), ("gn2_b", [D], F32)]:
        if fl(nm):
            opt[nm] = nc.declare_dram_parameter(nm, shape, dt, isOutput=False)
    seq_out = nc.declare_dram_parameter("seq_out", [R, D], F32, isOutput=True)
    graph_out = nc.declare_dram_parameter("graph_out", [R, D], F32, isOutput=True)

    NT = R // 128
    NB = R // 256

    with tile.TileContext(nc) as tc:
        with tc.tile_pool(name="dram", bufs=1, space="DRAM") as dram_pool, \
             tc.tile_pool(name="wpre", bufs=1) as wpre:
            sB_s = dram_pool.tile([R, D], F16)
            sB_g = dram_pool.tile([R, D], F16)
            sT_s = dram_pool.tile([128, 8, R], F16)
            sT_g = dram_pool.tile([128, 8, R], F16)

            # One w1 buffer shared by phases B and C (re-filled for C); the
            # seq-FFN w1 prefetches during phase A in ht-major chunks.
            w1_sb = wpre.tile([128, 8, HID], F16)
            for hs in range(8):
                nc.sync.dma_start(out=w1_sb[:, :, hs * 512:(hs + 1) * 512],
                                  in_=w1s[:, :, hs * 512:(hs + 1) * 512])

            # ---------------- Phase A: attention + LN1 ----------------
            with tc.tile_pool(name="singlesA", bufs=1) as singles, \
                 tc.tile_pool(name="workA", bufs=4) as work, \
                 tc.tile_pool(name="pstpA", bufs=3, space="PSUM") as pstp, \
                 tc.tile_pool(name="psmmA", bufs=5, space="PSUM") as psmm:

                wcs_sb = singles.tile([128, 8, D], F16)
                wcg_sb = singles.tile([128, 8, D], F16)
                for kt in range(8):
                    nc.sync.dma_start(out=wcs_sb[:, kt, :], in_=wcs[:, kt, :])
                    nc.sync.dma_start(out=wcg_sb[:, kt, :], in_=wcg[:, kt, :])
                ident16 = singles.tile([128, 128], F16)
                make_identity(nc, ident16)
                magic = singles.tile([128, 2], U32)
                nc.vector.memset(magic, MAGIC)
                ones16 = None
                opt_sb = {}
                if fl("bcs") or fl("bcg"):
                    ones16 = singles.tile([1, 128], F16)
                    nc.vector.memset(ones16, 1.0)
                    for nm in ("bcs", "bcg"):
                        if nm in opt:
                            opt_sb[nm] = singles.tile([1, D], F16, name=f"sb_{nm}")
                            nc.sync.dma_start(out=opt_sb[nm], in_=opt[nm][:, :])
                ln_bcs = {}
                for nm in ("sn1_g", "sn1_b", "gn1_g", "gn1_b"):
                    if nm in opt:
                        ln_bcs[nm] = _bcast_param(nc, singles, opt[nm].ap(), D, nm)

                for t in range(NT):
                    row = t * 128
                    S16 = work.tile([128, D], F16, tag="S16")
                    nc.sync.dma_start(out=S16, in_=seq16[row:row + 128, :])
                    G16 = work.tile([128, D], F16, tag="G16")
                    nc.sync.dma_start(out=G16, in_=graph16[row:row + 128, :])
                    ST = work.tile([128, 8, 128], F16, tag="ST")
                    nc.sync.dma_start(out=ST, in_=seqT[:, :, row:row + 128])
                    GT = work.tile([128, 8, 128], F16, tag="GT")
                    nc.sync.dma_start(out=GT, in_=graphT[:, :, row:row + 128])

                    x_s = work.tile([128, D], F16, tag="xs")
                    x_g = work.tile([128, D], F16, tag="xg")
                    # interleave the four attention psum tiles; evacuate each
                    # with its residual add as soon as its group completes so
                    # the next tile's matmuls aren't starved of PSUM.
                    for half in range(2):
                        nsl = slice(half * 512, (half + 1) * 512)
                        ga = psmm.tile([128, 512], F32, tag="attnps",
                                       name=f"ga{t}_{half}")
                        for kt in range(8):
                            nc.tensor.matmul(ga, lhsT=ST[:, kt, :],
                                             rhs=wcs_sb[:, kt, nsl],
                                             start=(kt == 0),
                                             stop=(kt == 7 and not fl("bcs")))
                        if fl("bcs"):
                            nc.tensor.matmul(ga, lhsT=ones16,
                                             rhs=opt_sb["bcs"][:, nsl],
                                             start=False, stop=True)
                        nc.vector.tensor_add(out=x_g[:, nsl], in0=G16[:, nsl],
                                             in1=ga)
                        sa = psmm.tile([128, 512], F32, tag="attnps",
                                       name=f"sa{t}_{half}")
                        for kt in range(8):
                            nc.tensor.matmul(sa, lhsT=GT[:, kt, :],
                                             rhs=wcg_sb[:, kt, nsl],
                                             start=(kt == 0),
                                             stop=(kt == 7 and not fl("bcg")))
                        if fl("bcg"):
                            nc.tensor.matmul(sa, lhsT=ones16,
                                             rhs=opt_sb["bcg"][:, nsl],
                                             start=False, stop=True)
                        nc.vector.tensor_add(out=x_s[:, nsl], in0=S16[:, nsl],
                                             in1=sa)

                    res16_s = work.tile([128, D], F16, tag="res16s")
                    res16_g = work.tile([128, D], F16, tag="res16g")
                    if ln_bcs:
                        for x, res16, g_nm, b_nm in (
                            (x_s, res16_s, "sn1_g", "sn1_b"),
                            (x_g, res16_g, "gn1_g", "gn1_b"),
                        ):
                            _ln_tail(nc, work, magic[:, 0:1], x, res16,
                                     ln_bcs.get(g_nm), ln_bcs.get(b_nm))
                    else:
                        _ln_dual_fast(nc, work, magic, x_s, x_g,
                                      res16_s, res16_g)

                    for which, res16, sB_d, sT_d in (
                        ("s", res16_s, sB_s, sT_s),
                        ("g", res16_g, sB_g, sT_g),
                    ):
                        rTt = work.tile([128, 8, 128], F16, tag=f"rTt{which}")
                        for grp in range(2):
                            # padded to a full PSUM bank to avoid bank-sharing
                            # serialization between transpose writes and the
                            # evacuation reads of the neighbouring buffer.
                            tpr = pstp.tile([128, 1024], F16, tag="tp",
                                            name=f"tpr{which}{t}_{grp}")
                            for j in range(4):
                                kt = grp * 4 + j
                                nc.tensor.transpose(tpr[:, j * 128:(j + 1) * 128],
                                                    res16[:, kt * 128:(kt + 1) * 128],
                                                    ident16)
                            nc.vector.tensor_copy(
                                out=rTt[:, grp * 4:(grp + 1) * 4, :].rearrange("p a b -> p (a b)"),
                                in_=tpr[:, 0:512])
                        nc.sync.dma_start(out=sT_d[:, :, row:row + 128], in_=rTt)
                        nc.sync.dma_start(out=sB_d[row:row + 128, :], in_=res16)

            # ---------------- Phases B/C: FFN + LN2 ----------------
            for ph, (w1_in, w2_in, sB_d, sT_d, out_d, b1_nm, b2_nm, g_nm, b_nm) in enumerate((
                (w1s, w2s, sB_s, sT_s, seq_out, "b1s", "b2s", "sn2_g", "sn2_b"),
                (w1g, w2g, sB_g, sT_g, graph_out, "b1g", "b2g", "gn2_g", "gn2_b"),
            )):
                with tc.tile_pool(name=f"singles{ph}", bufs=1) as singles, \
                     tc.tile_pool(name=f"work{ph}", bufs=3) as work, \
                     tc.tile_pool(name=f"hg{ph}", bufs=8) as hgpool, \
                     tc.tile_pool(name=f"psh{ph}", bufs=3, space="PSUM") as psh, \
                     tc.tile_pool(name=f"pso{ph}", bufs=4, space="PSUM") as pso:

                    # stage the first block's activations ahead of the bulk
                    # weight DMA so the phase's first matmuls aren't queued
                    # behind 8+ MB of weight traffic.
                    rT0 = work.tile([128, 8, 256], F16, tag="rT")
                    nc.sync.dma_start(out=rT0, in_=sT_d[:, :, 0:256])
                    rB0 = work.tile([128, 2, D], F16, tag="rB")
                    nc.sync.dma_start(
                        out=rB0,
                        in_=sB_d[0:256, :].rearrange("(s p) n -> p s n", p=128))
                    if ph != 0:
                        # refill the shared w1 buffer for the graph FFN;
                        # dependency tracking delays each chunk's DMA until
                        # phase B's reads of that region have completed.
                        for hs in range(8):
                            nc.sync.dma_start(
                                out=w1_sb[:, :, hs * 512:(hs + 1) * 512],
                                in_=w1_in[:, :, hs * 512:(hs + 1) * 512])
                    w2_sb = singles.tile([128, 32, D], F16)
                    for ktg in range(8):
                        nc.sync.dma_start(out=w2_sb[:, ktg * 4:(ktg + 1) * 4, :],
                                          in_=w2_in[:, ktg * 4:(ktg + 1) * 4, :])
                    magic = singles.tile([128, 1], U32)
                    nc.vector.memset(magic, MAGIC)
                    b1_sb = None
                    if b1_nm in opt:
                        b1_sb = singles.tile([128, 32], F32)
                        nc.sync.dma_start(out=b1_sb, in_=opt[b1_nm][:, :])
                    ones16 = None
                    b2_sb = None
                    if b2_nm in opt:
                        ones16 = singles.tile([1, 128], F16)
                        nc.vector.memset(ones16, 1.0)
                        b2_sb = singles.tile([1, D], F16)
                        nc.sync.dma_start(out=b2_sb, in_=opt[b2_nm][:, :])
                    ln_g_bc = (_bcast_param(nc, singles, opt[g_nm].ap(), D, g_nm)
                               if g_nm in opt else None)
                    ln_b_bc = (_bcast_param(nc, singles, opt[b_nm].ap(), D, b_nm)
                               if b_nm in opt else None)

                    for blk in range(NB):
                        brow = blk * 256
                        if blk == 0:
                            rT, rB = rT0, rB0
                        else:
                            rT = work.tile([128, 8, 256], F16, tag="rT")
                            nc.sync.dma_start(out=rT, in_=sT_d[:, :, brow:brow + 256])
                            rB = work.tile([128, 2, D], F16, tag="rB")
                            nc.sync.dma_start(
                                out=rB,
                                in_=sB_d[brow:brow + 256, :].rearrange("(s p) n -> p s n", p=128))
                        ops = [pso.tile([128, 512], F32, tag="ops", name=f"ops{blk}_{_h}")
                               for _h in range(4)]
                        for ht in range(32):
                            # full-bank psum tile (2 KiB) so consecutive hps
                            # buffers never share a PSUM bank — a shared bank
                            # serializes the gelu read against the next
                            # mm1's writes and stalls the whole pipeline.
                            hps = psh.tile([128, 512], F32, tag="hps")
                            for kt in range(8):
                                nc.tensor.matmul(hps[:, 0:256],
                                                 lhsT=w1_sb[:, kt, ht * 128:(ht + 1) * 128],
                                                 rhs=rT[:, kt, :],
                                                 start=(kt == 0), stop=(kt == 7))
                            hg = hgpool.tile([128, 256], F16, tag="hg")
                            if b1_sb is not None:
                                nc.scalar.activation(out=hg, in_=hps[:, 0:256],
                                                     func=AF.Gelu,
                                                     bias=b1_sb[:, ht:ht + 1],
                                                     scale=1.0, alpha=0.0)
                            else:
                                nc.scalar.activation(out=hg, in_=hps[:, 0:256],
                                                     func=AF.Gelu)
                            for bs in range(2):
                                for nh in range(2):
                                    nc.tensor.matmul(
                                        ops[bs * 2 + nh],
                                        lhsT=hg[:, bs * 128:(bs + 1) * 128],
                                        rhs=w2_sb[:, ht, nh * 512:(nh + 1) * 512],
                                        start=(ht == 0),
                                        stop=(ht == 31 and b2_sb is None))
                        if b2_sb is not None:
                            for bs in range(2):
                                for nh in range(2):
                                    nc.tensor.matmul(ops[bs * 2 + nh], lhsT=ones16,
                                                     rhs=b2_sb[:, nh * 512:(nh + 1) * 512],
                                                     start=False, stop=True)
                        for bs in range(2):
                            x2 = work.tile([128, D], F32, tag="x2")
                            nc.vector.tensor_add(out=x2[:, 0:512], in0=rB[:, bs, 0:512],
                                                 in1=ops[bs * 2 + 0])
                            nc.vector.tensor_add(out=x2[:, 512:1024],
                                                 in0=rB[:, bs, 512:1024],
                                                 in1=ops[bs * 2 + 1])
                            ot = work.tile([128, D], F32, tag="ot")
                            _ln_tail(nc, work, magic, x2, ot, ln_g_bc, ln_b_bc)
                            nc.sync.dma_start(
                                out=out_d[brow + bs * 128:brow + bs * 128 + 128, :],
                                in_=ot)

    nc.compile()
    return nc


def _host_prep(inputs):
    f = lambda k: np.asarray(inputs[k])
    flags = {}

    def fold(pfx):
        in_w = f(f"{pfx}_in_w").astype(np.float64)
        in_b = f(f"{pfx}_in_b").astype(np.float64)
        out_w = f(f"{pfx}_out_w").astype(np.float64)
        out_b = f(f"{pfx}_out_b").astype(np.float64)
        Wc = out_w @ in_w[2 * D:]
        bc = in_b[2 * D:] @ out_w.T + out_b
        return Wc, bc

    Wcs, bcs = fold("s2g")
    Wcg, bcg = fold("g2s")

    def rhs_tiles(W, kt):  # W [n, d_in] -> [128, kt, n] f16 tiles of W.T
        return np.ascontiguousarray(
            W.T.reshape(kt, 128, -1).transpose(1, 0, 2)).astype(np.float16)

    wm = {
        "wcs": rhs_tiles(Wcs, 8), "wcg": rhs_tiles(Wcg, 8),
        "w1s": rhs_tiles(f("seq_w1"), 8), "w2s": rhs_tiles(f("seq_w2"), 32),
        "w1g": rhs_tiles(f("gr_w1"), 8), "w2g": rhs_tiles(f("gr_w2"), 32),
    }
    if np.any(bcs != 0):
        flags["bcs"] = True
        wm["bcs"] = bcs.astype(np.float16).reshape(1, D)
    if np.any(bcg != 0):
        flags["bcg"] = True
        wm["bcg"] = bcg.astype(np.float16).reshape(1, D)
    for nm, key in (("b1s", "seq_b1"), ("b1g", "gr_b1")):
        v = f(key)
        if np.any(v != 0):
            flags[nm] = True
            wm[nm] = np.ascontiguousarray(v.reshape(32, 128).T).astype(np.float32)
    for nm, key in (("b2s", "seq_b2"), ("b2g", "gr_b2")):
        v = f(key)
        if np.any(v != 0):
            flags[nm] = True
            wm[nm] = v.astype(np.float16).reshape(1, D)
    for nm in ("sn1", "sn2", "gn1", "gn2"):
        g = f(f"{nm}_g"); b = f(f"{nm}_b")
        if np.any(g != 1):
            flags[f"{nm}_g"] = True
            wm[f"{nm}_g"] = g.astype(np.float32)
        if np.any(b != 0):
            flags[f"{nm}_b"] = True
            wm[f"{nm}_b"] = b.astype(np.float32)

    seq = np.asarray(f("seq_emb"), dtype=np.float32)
    graph = np.asarray(f("graph_emb"), dtype=np.float32)
    seq16_full = seq.astype(np.float16)
    graph16_full = graph.astype(np.float16)
    in_maps = []
    for i in range(N_CORES):
        m = dict(wm)
        s = seq16_full[i * R:(i + 1) * R]
        g = graph16_full[i * R:(i + 1) * R]
        m["seq16"] = np.ascontiguousarray(s)
        m["graph16"] = np.ascontiguousarray(g)
        m["seqT"] = rhs_tiles(s, 8)
        m["graphT"] = rhs_tiles(g, 8)
        in_maps.append(m)
    return in_maps, flags


def kernel(**inputs):
    in_maps, flags = _host_prep(inputs)
    key = tuple(sorted(flags.items()))
    if key not in _cache:
        _cache[key] = _build(flags)
    nc = _cache[key]
    res = run_bass_kernel_spmd(nc, in_maps, core_ids=list(range(N_CORES)))
    seq_out = np.concatenate([res.results[i]["seq_out"] for i in range(N_CORES)], axis=0)
    graph_out = np.concatenate([res.results[i]["graph_out"] for i in range(N_CORES)], axis=0)
    return (seq_out, graph_out)


# revision 19
# speedup vs baseline: 1.2394x; 1.0657x over previous
"""Trainium2 Bass kernel for nn_BimodalCrossAttentionBlock.

Math: seq-len-1 multihead cross attention => softmax over a single key is
identically 1, so MHA(x_q, x_kv) collapses to out_proj(v_proj(x_kv)) and the
two projections fold into one matrix Wc = out_w @ in_w[2D:] (Q/K projections
and num_heads are dead).  The block then is:
  graph_res = LN(graph + seq @ Wc_s2g.T + bc_s2g)     (gn1)
  seq_res   = LN(seq + graph @ Wc_g2s.T + bc_g2s)     (sn1)
  seq_out   = LN(seq_res + FFN_seq(seq_res))          (sn2)
  graph_out = LN(graph_res + FFN_gr(graph_res))       (gn2)

Sharding: pure data parallel over the batch dim (4096 rows/core x 8 cores);
weights replicated, no collectives.  Matmuls run in fp16 (fp32 PSUM accum),
skip paths / LayerNorm in fp32.  Host supplies the inputs in f16 in BOTH
batch-major and feature-major (pre-transposed) layouts, so phase A needs no
on-device input transposes.  Three on-device phases:
  A: attention + LN1 for both modalities (activation-stationary matmuls,
     batch-major outputs; LN rsqrt via Newton iteration on DVE); emits res in
     batch-major f16 (skip path) and feature-major f16 (FFN operand).
  B: seq FFN + LN2 (w1 prefetched during phase A; w2 streamed in ht-major
     chunks so the first matmuls start immediately)
  C: graph FFN + LN2
"""
import numpy as np

import concourse.bass as bass
import concourse.bacc as bacc
import concourse.tile as tile
import concourse.mybir as mybir
from concourse.bass_utils import run_bass_kernel_spmd
from concourse.masks import make_identity

F16 = mybir.dt.float16
F32 = mybir.dt.float32
U32 = mybir.dt.uint32
AF = mybir.ActivationFunctionType
ALU = mybir.AluOpType

N_CORES = 8
B_FULL = 32768
D = 1024
HID = 4096
R = B_FULL // N_CORES
EPS = 1e-5
MAGIC = 0x5F3759DF

_cache = {}


def _ln_tail(nc, work, magic, x2, out_tile, lng_bc, lnb_bc):
    """LayerNorm of x2 [128, D] f32 -> out_tile; stats + rsqrt all on DVE."""
    stats = work.tile([128, 2, 6], F32, tag="lnstats")
    mv = work.tile([128, 2], F32, tag="lnmv")
    nc.vector.bn_stats(out=stats[:, 0, :], in_=x2[:, 0:512])
    nc.vector.bn_stats(out=stats[:, 1, :], in_=x2[:, 512:1024])
    nc.vector.bn_aggr(out=mv, in_=stats)
    v = work.tile([128, 1], F32, tag="lnv")
    nc.vector.tensor_scalar(out=v, in0=mv[:, 1:2], scalar1=EPS, scalar2=None,
                            op0=ALU.add)
    y = work.tile([128, 1], F32, tag="lny")
    t = work.tile([128, 1], F32, tag="lnt")
    nc.vector.tensor_scalar(out=y.bitcast(U32), in0=v.bitcast(U32), scalar1=1,
                            scalar2=None, op0=ALU.logical_shift_right)
    nc.vector.tensor_tensor(out=y.bitcast(U32), in0=magic, in1=y.bitcast(U32),
                            op=ALU.subtract)
    for _ in range(3):
        nc.vector.tensor_mul(out=t, in0=y, in1=y)
        nc.vector.tensor_mul(out=t, in0=t, in1=v)
        nc.vector.tensor_scalar(out=t, in0=t, scalar1=-0.5, scalar2=1.5,
                                op0=ALU.mult, op1=ALU.add)
        nc.vector.tensor_mul(out=y, in0=y, in1=t)
    if lng_bc is None and lnb_bc is None:
        nc.vector.tensor_scalar(out=out_tile, in0=x2, scalar1=mv[:, 0:1],
                                scalar2=y, op0=ALU.subtract, op1=ALU.mult)
    else:
        tmp = work.tile([128, 1024], F32, tag="lntmp")
        nc.vector.tensor_scalar(out=tmp, in0=x2, scalar1=mv[:, 0:1],
                                scalar2=y, op0=ALU.subtract, op1=ALU.mult)
        if lng_bc is not None:
            nc.vector.tensor_mul(out=tmp, in0=tmp, in1=lng_bc)
        if lnb_bc is not None:
            nc.vector.tensor_add(out=out_tile, in0=tmp, in1=lnb_bc)
        else:
            nc.vector.tensor_copy(out=out_tile, in_=tmp)


def _ln_dual_fast(nc, work, magic, x_s, x_g, res_s, res_g):
    """LayerNorm (no affine) of two [128, D] tiles; stats on DVE, the final
    normalize on the scalar engine (out = Identity(x * rstd - mu*rstd))."""
    stats = work.tile([128, 2, 2, 6], F32, tag="lnstats2")
    mv = work.tile([128, 2, 2], F32, tag="lnmv2")
    for i, x in enumerate((x_s, x_g)):
        nc.vector.bn_stats(out=stats[:, i, 0, :], in_=x[:, 0:512])
        nc.vector.bn_stats(out=stats[:, i, 1, :], in_=x[:, 512:1024])
        nc.vector.bn_aggr(out=mv[:, i, :], in_=stats[:, i, :, :])
    v = work.tile([128, 2], F32, tag="lnv2")
    nc.vector.tensor_scalar(out=v, in0=mv[:, :, 1], scalar1=EPS, scalar2=None,
                            op0=ALU.add)
    y = work.tile([128, 2], F32, tag="lny2")
    t = work.tile([128, 2], F32, tag="lnt2")
    nc.vector.tensor_scalar(out=y.bitcast(U32), in0=v.bitcast(U32), scalar1=1,
                            scalar2=None, op0=ALU.logical_shift_right)
    nc.vector.tensor_tensor(out=y.bitcast(U32), in0=magic, in1=y.bitcast(U32),
                            op=ALU.subtract)
    for _ in range(3):
        nc.vector.tensor_mul(out=t, in0=y, in1=y)
        nc.vector.tensor_mul(out=t, in0=t, in1=v)
        nc.vector.tensor_scalar(out=t, in0=t, scalar1=-0.5, scalar2=1.5,
                                op0=ALU.mult, op1=ALU.add)
        nc.vector.tensor_mul(out=y, in0=y, in1=t)
    nmy = work.tile([128, 2], F32, tag="lnnmy2")
    nc.vector.tensor_tensor(out=nmy, in0=mv[:, :, 0], in1=y, op=ALU.mult)
    nc.vector.tensor_scalar(out=nmy, in0=nmy, scalar1=-1.0, scalar2=None,
                            op0=ALU.mult)
    for i, (x, res) in enumerate(((x_s, res_s), (x_g, res_g))):
        nc.scalar.activation(out=res, in_=x, func=AF.Identity,
                             scale=y[:, i:i + 1], bias=nmy[:, i:i + 1])


def _bcast_param(nc, pool, dram_ap, n, tag):
    t = pool.tile([128, n], F32, tag=tag)
    src = bass.AP(tensor=dram_ap.tensor, offset=dram_ap.offset,
                  ap=[[0, 128]] + dram_ap.ap)
    nc.gpsimd.dma_start(out=t, in_=src)
    return t


def _build(flags):
    fl = lambda k: bool(flags.get(k, False))
    nc = bacc.Bacc("TRN2", target_bir_lowering=False, debug=False,
                   num_devices=N_CORES)

    seq16 = nc.declare_dram_parameter("seq16", [R, D], F16, isOutput=False)
    graph16 = nc.declare_dram_parameter("graph16", [R, D], F16, isOutput=False)
    seqT = nc.declare_dram_parameter("seqT", [128, 8, R], F16, isOutput=False)
    graphT = nc.declare_dram_parameter("graphT", [128, 8, R], F16, isOutput=False)
    wcs = nc.declare_dram_parameter("wcs", [128, 8, D], F16, isOutput=False)
    wcg = nc.declare_dram_parameter("wcg", [128, 8, D], F16, isOutput=False)
    w1s = nc.declare_dram_parameter("w1s", [128, 8, HID], F16, isOutput=False)
    w2s = nc.declare_dram_parameter("w2s", [128, 32, D], F16, isOutput=False)
    w1g = nc.declare_dram_parameter("w1g", [128, 8, HID], F16, isOutput=False)
    w2g = nc.declare_dram_parameter("w2g", [128, 32, D], F16, isOutput=False)
    opt = {}
    for nm, shape, dt in [("bcs", [1, D], F16), ("bcg", [1, D], F16),
                          ("b1s", [128, 32], F32), ("b1g", [128, 32], F32),
                          ("b2s", [1, D], F16), ("b2g", [1, D], F16),
                          ("sn1_g", [D], F32), ("sn1_b", [D], F32),
                          ("sn2_g", [D], F32), ("sn2_b", [D], F32),
                          ("gn1_g", [D], F32), ("gn1_b", [D], F32),
                          ("gn2_g", [D], F32), ("gn2_b", [D], F32)]:
        if fl(nm):
            opt[nm] = nc.declare_dram_parameter(nm, shape, dt, isOutput=False)
    seq_out = nc.declare_dram_parameter("seq_out", [R, D], F32, isOutput=True)
    graph_out = nc.declare_dram_parameter("graph_out", [R, D], F32, isOutput=True)

    NT = R // 128
    NB = R // 256

    with tile.TileContext(nc) as tc:
        with tc.tile_pool(name="dram", bufs=1, space="DRAM") as dram_pool, \
             tc.tile_pool(name="wpre", bufs=1) as wpre:
            sB_s = dram_pool.tile([R, D], F16)
            sB_g = dram_pool.tile([R, D], F16)
            sT_s = dram_pool.tile([128, 8, R], F16)
            sT_g = dram_pool.tile([128, 8, R], F16)

            # One w1 buffer shared by phases B and C (re-filled for C); the
            # seq-FFN w1 prefetches during phase A in ht-major chunks.
            w1_sb = wpre.tile([128, 8, HID], F16)
            for hs in range(8):
                nc.sync.dma_start(out=w1_sb[:, :, hs * 512:(hs + 1) * 512],
                                  in_=w1s[:, :, hs * 512:(hs + 1) * 512])

            # ---------------- Phase A: attention + LN1 ----------------
            with tc.tile_pool(name="singlesA", bufs=1) as singles, \
                 tc.tile_pool(name="workA", bufs=4) as work, \
                 tc.tile_pool(name="pstpA", bufs=3, space="PSUM") as pstp, \
                 tc.tile_pool(name="psmmA", bufs=5, space="PSUM") as psmm:

                wcs_sb = singles.tile([128, 8, D], F16)
                wcg_sb = singles.tile([128, 8, D], F16)
                for kt in range(8):
                    nc.sync.dma_start(out=wcs_sb[:, kt, :], in_=wcs[:, kt, :])
                    nc.sync.dma_start(out=wcg_sb[:, kt, :], in_=wcg[:, kt, :])
                ident16 = singles.tile([128, 128], F16)
                make_identity(nc, ident16)
                magic = singles.tile([128, 2], U32)
                nc.vector.memset(magic, MAGIC)
                ones16 = None
                opt_sb = {}
                if fl("bcs") or fl("bcg"):
                    ones16 = singles.tile([1, 128], F16)
                    nc.vector.memset(ones16, 1.0)
                    for nm in ("bcs", "bcg"):
                        if nm in opt:
                            opt_sb[nm] = singles.tile([1, D], F16, name=f"sb_{nm}")
                            nc.sync.dma_start(out=opt_sb[nm], in_=opt[nm][:, :])
                ln_bcs = {}
                for nm in ("sn1_g", "sn1_b", "gn1_g", "gn1_b"):
                    if nm in opt:
                        ln_bcs[nm] = _bcast_param(nc, singles, opt[nm].ap(), D, nm)

                for t in range(NT):
                    row = t * 128
                    S16 = work.tile([128, D], F16, tag="S16")
                    nc.sync.dma_start(out=S16, in_=seq16[row:row + 128, :])
                    G16 = work.tile([128, D], F16, tag="G16")
                    nc.sync.dma_start(out=G16, in_=graph16[row:row + 128, :])
                    ST = work.tile([128, 8, 128], F16, tag="ST")
                    nc.sync.dma_start(out=ST, in_=seqT[:, :, row:row + 128])
                    GT = work.tile([128, 8, 128], F16, tag="GT")
                    nc.sync.dma_start(out=GT, in_=graphT[:, :, row:row + 128])

                    x_s = work.tile([128, D], F16, tag="xs")
                    x_g = work.tile([128, D], F16, tag="xg")
                    # interleave the four attention psum tiles; evacuate each
                    # with its residual add as soon as its group completes so
                    # the next tile's matmuls aren't starved of PSUM.
                    for half in range(2):
                        nsl = slice(half * 512, (half + 1) * 512)
                        ga = psmm.tile([128, 512], F32, tag="attnps",
                                       name=f"ga{t}_{half}")
                        for kt in range(8):
                            nc.tensor.matmul(ga, lhsT=ST[:, kt, :],
                                             rhs=wcs_sb[:, kt, nsl],
                                             start=(kt == 0),
                                             stop=(kt == 7 and not fl("bcs")))
                        if fl("bcs"):
                            nc.tensor.matmul(ga, lhsT=ones16,
                                             rhs=opt_sb["bcs"][:, nsl],
                                             start=False, stop=True)
                        nc.vector.tensor_add(out=x_g[:, nsl], in0=G16[:, nsl],
                                             in1=ga)
                        sa = psmm.tile([128, 512], F32, tag="attnps",
                                       name=f"sa{t}_{half}")
                        for kt in range(8):
                            nc.tensor.matmul(sa, lhsT=GT[:, kt, :],
                                             rhs=wcg_sb[:, kt, nsl],
                                             start=(kt == 0),
                                             stop=(kt == 7 and not fl("bcg")))
                        if fl("bcg"):
                            nc.tensor.matmul(sa, lhsT=ones16,
                                             rhs=opt_sb["bcg"][:, nsl],
                                             start=False, stop=True)
                        nc.vector.tensor_add(out=x_s[:, nsl], in0=S16[:, nsl],
                                             in1=sa)

                    res16_s = work.tile([128, D], F16, tag="res16s")
                    res16_g = work.tile([128, D], F16, tag="res16g")
                    if ln_bcs:
                        for x, res16, g_nm, b_nm in (
                            (x_s, res16_s, "sn1_g", "sn1_b"),
                            (x_g, res16_g, "gn1_g", "gn1_b"),
                        ):
                            _ln_tail(nc, work, magic[:, 0:1], x, res16,
                                     ln_bcs.get(g_nm), ln_bcs.get(b_nm))
                    else:
                        _ln_dual_fast(nc, work, magic, x_s, x_g,
                                      res16_s, res16_g)

                    for which, res16, sB_d, sT_d in (
                        ("s", res16_s, sB_s, sT_s),
                        ("g", res16_g, sB_g, sT_g),
                    ):
                        rTt = work.tile([128, 8, 128], F16, tag=f"rTt{which}")
                        for grp in range(2):
                            # padded to a full PSUM bank to avoid bank-sharing
                            # serialization between transpose writes and the
                            # evacuation reads of the neighbouring buffer.
                            tpr = pstp.tile([128, 1024], F16, tag="tp",
                                            name=f"tpr{which}{t}_{grp}")
                            for j in range(4):
                                kt = grp * 4 + j
                                nc.tensor.transpose(tpr[:, j * 128:(j + 1) * 128],
                                                    res16[:, kt * 128:(kt + 1) * 128],
                                                    ident16)
                            nc.vector.tensor_copy(
                                out=rTt[:, grp * 4:(grp + 1) * 4, :].rearrange("p a b -> p (a b)"),
                                in_=tpr[:, 0:512])
                        nc.sync.dma_start(out=sT_d[:, :, row:row + 128], in_=rTt)
                        nc.sync.dma_start(out=sB_d[row:row + 128, :], in_=res16)

            # ---------------- Phases B/C: FFN + LN2 ----------------
            for ph, (w1_in, w2_in, sB_d, sT_d, out_d, b1_nm, b2_nm, g_nm, b_nm) in enumerate((
                (w1s, w2s, sB_s, sT_s, seq_out, "b1s", "b2s", "sn2_g", "sn2_b"),
                (w1g, w2g, sB_g, sT_g, graph_out, "b1g", "b2g", "gn2_g", "gn2_b"),
            )):
                with tc.tile_pool(name=f"singles{ph}", bufs=1) as singles, \
                     tc.tile_pool(name=f"work{ph}", bufs=3) as work, \
                     tc.tile_pool(name=f"hg{ph}", bufs=8) as hgpool, \
                     tc.tile_pool(name=f"psh{ph}", bufs=3, space="PSUM") as psh, \
                     tc.tile_pool(name=f"pso{ph}", bufs=4, space="PSUM") as pso:

                    # stage the first block's activations ahead of the bulk
                    # weight DMA so the phase's first matmuls aren't queued
                    # behind 8+ MB of weight traffic.
                    rT0 = work.tile([128, 8, 256], F16, tag="rT")
                    nc.sync.dma_start(out=rT0, in_=sT_d[:, :, 0:256])
                    rB0 = work.tile([128, 2, D], F16, tag="rB")
                    nc.sync.dma_start(
                        out=rB0,
                        in_=sB_d[0:256, :].rearrange("(s p) n -> p s n", p=128))
                    if ph != 0:
                        # refill the shared w1 buffer for the graph FFN;
                        # dependency tracking delays each chunk's DMA until
                        # phase B's reads of that region have completed.
                        for hs in range(8):
                            nc.sync.dma_start(
                                out=w1_sb[:, :, hs * 512:(hs + 1) * 512],
                                in_=w1_in[:, :, hs * 512:(hs + 1) * 512])
                    w2_sb = singles.tile([128, 32, D], F16)
                    for ktg in range(8):
                        nc.sync.dma_start(out=w2_sb[:, ktg * 4:(ktg + 1) * 4, :],
                                          in_=w2_in[:, ktg * 4:(ktg + 1) * 4, :])
                    magic = singles.tile([128, 1], U32)
                    nc.vector.memset(magic, MAGIC)
                    b1_sb = None
                    if b1_nm in opt:
                        b1_sb = singles.tile([128, 32], F32)
                        nc.sync.dma_start(out=b1_sb, in_=opt[b1_nm][:, :])
                    ones16 = None
                    b2_sb = None
                    if b2_nm in opt:
                        ones16 = singles.tile([1, 128], F16)
                        nc.vector.memset(ones16, 1.0)
                        b2_sb = singles.tile([1, D], F16)
                        nc.sync.dma_start(out=b2_sb, in_=opt[b2_nm][:, :])
                    ln_g_bc = (_bcast_param(nc, singles, opt[g_nm].ap(), D, g_nm)
                               if g_nm in opt else None)
                    ln_b_bc = (_bcast_param(nc, singles, opt[b_nm].ap(), D, b_nm)
                               if b_nm in opt else None)

                    for blk in range(NB):
                        brow = blk * 256
                        if blk == 0:
                            rT, rB = rT0, rB0
                        else:
                            rT = work.tile([128, 8, 256], F16, tag="rT")
                            nc.sync.dma_start(out=rT, in_=sT_d[:, :, brow:brow + 256])
                            rB = work.tile([128, 2, D], F16, tag="rB")
                            nc.sync.dma_start(
                                out=rB,
                                in_=sB_d[brow:brow + 256, :].rearrange("(s p) n -> p s n", p=128))
                        ops = [pso.tile([128, 512], F32, tag="ops", name=f"ops{blk}_{_h}")
                               for _h in range(4)]
                        # software-pipelined: mm2 for tile ht is emitted one
                        # iteration later, so its gelu output has been ready
                        # for a full cycle when the matmuls issue (otherwise
                        # the first mm2 stalls ~140 ns on the weight load).
                        def emit_mm2(hg_p, ht_p):
                            for bs in range(2):
                                for nh in range(2):
                                    nc.tensor.matmul(
                                        ops[bs * 2 + nh],
                                        lhsT=hg_p[:, bs * 128:(bs + 1) * 128],
                                        rhs=w2_sb[:, ht_p, nh * 512:(nh + 1) * 512],
                                        start=(ht_p == 0),
                                        stop=(ht_p == 31 and b2_sb is None))

                        pipe = None
                        for ht in range(32):
                            # full-bank psum tile (2 KiB) so consecutive hps
                            # buffers never share a PSUM bank — a shared bank
                            # serializes the gelu read against the next
                            # mm1's writes and stalls the whole pipeline.
                            hps = psh.tile([128, 512], F32, tag="hps")
                            for kt in range(8):
                                nc.tensor.matmul(hps[:, 0:256],
                                                 lhsT=w1_sb[:, kt, ht * 128:(ht + 1) * 128],
                                                 rhs=rT[:, kt, :],
                                                 start=(kt == 0), stop=(kt == 7))
                            hg = hgpool.tile([128, 256], F16, tag="hg")
                            if b1_sb is not None:
                                nc.scalar.activation(out=hg, in_=hps[:, 0:256],
                                                     func=AF.Gelu,
                                                     bias=b1_sb[:, ht:ht + 1],
                                                     scale=1.0, alpha=0.0)
                            else:
                                nc.scalar.activation(out=hg, in_=hps[:, 0:256],
                                                     func=AF.Gelu)
                            if pipe is not None:
                                emit_mm2(*pipe)
                            pipe = (hg, ht)
                        emit_mm2(*pipe)
                        if b2_sb is not None:
                            for bs in range(2):
                                for nh in range(2):
                                    nc.tensor.matmul(ops[bs * 2 + nh], lhsT=ones16,
                                                     rhs=b2_sb[:, nh * 512:(nh + 1) * 512],
                                                     start=False, stop=True)
                        for bs in range(2):
                            x2 = work.tile([128, D], F32, tag="x2")
                            nc.vector.tensor_add(out=x2[:, 0:512], in0=rB[:, bs, 0:512],
                                                 in1=ops[bs * 2 + 0])
                            nc.vector.tensor_add(out=x2[:, 512:1024],
                                                 in0=rB[:, bs, 512:1024],
                                                 in1=ops[bs * 2 + 1])
                            ot = work.tile([128, D], F32, tag="ot")
                            _ln_tail(nc, work, magic, x2, ot, ln_g_bc, ln_b_bc)
                            nc.sync.dma_start(
                                out=out_d[brow + bs * 128:brow + bs * 128 + 128, :],
                                in_=ot)

    nc.compile()
    return nc


def _host_prep(inputs):
    f = lambda k: np.asarray(inputs[k])
    flags = {}

    def fold(pfx):
        in_w = f(f"{pfx}_in_w").astype(np.float64)
        in_b = f(f"{pfx}_in_b").astype(np.float64)
        out_w = f(f"{pfx}_out_w").astype(np.float64)
        out_b = f(f"{pfx}_out_b").astype(np.float64)
        Wc = out_w @ in_w[2 * D:]
        bc = in_b[2 * D:] @ out_w.T + out_b
        return Wc, bc

    Wcs, bcs = fold("s2g")
    Wcg, bcg = fold("g2s")

    def rhs_tiles(W, kt):  # W [n, d_in] -> [128, kt, n] f16 tiles of W.T
        return np.ascontiguousarray(
            W.T.reshape(kt, 128, -1).transpose(1, 0, 2)).astype(np.float16)

    wm = {
        "wcs": rhs_tiles(Wcs, 8), "wcg": rhs_tiles(Wcg, 8),
        "w1s": rhs_tiles(f("seq_w1"), 8), "w2s": rhs_tiles(f("seq_w2"), 32),
        "w1g": rhs_tiles(f("gr_w1"), 8), "w2g": rhs_tiles(f("gr_w2"), 32),
    }
    if np.any(bcs != 0):
        flags["bcs"] = True
        wm["bcs"] = bcs.astype(np.float16).reshape(1, D)
    if np.any(bcg != 0):
        flags["bcg"] = True
        wm["bcg"] = bcg.astype(np.float16).reshape(1, D)
    for nm, key in (("b1s", "seq_b1"), ("b1g", "gr_b1")):
        v = f(key)
        if np.any(v != 0):
            flags[nm] = True
            wm[nm] = np.ascontiguousarray(v.reshape(32, 128).T).astype(np.float32)
    for nm, key in (("b2s", "seq_b2"), ("b2g", "gr_b2")):
        v = f(key)
        if np.any(v != 0):
            flags[nm] = True
            wm[nm] = v.astype(np.float16).reshape(1, D)
    for nm in ("sn1", "sn2", "gn1", "gn2"):
        g = f(f"{nm}_g"); b = f(f"{nm}_b")
        if np.any(g != 1):
            flags[f"{nm}_g"] = True
            wm[f"{nm}_g"] = g.astype(np.float32)
        if np.any(b != 0):
            flags[f"{nm}_b"] = True
            wm[f"{nm}_b"] = b.astype(np.float32)

    seq = np.asarray(f("seq_emb"), dtype=np.float32)
    graph = np.asarray(f("graph_emb"), dtype=np.float32)
    seq16_full = seq.astype(np.float16)
    graph16_full = graph.astype(np.float16)
    in_maps = []
    for i in range(N_CORES):
        m = dict(wm)
        s = seq16_full[i * R:(i + 1) * R]
        g = graph16_full[i * R:(i + 1) * R]
        m["seq16"] = np.ascontiguousarray(s)
        m["graph16"] = np.ascontiguousarray(g)
        m["seqT"] = rhs_tiles(s, 8)
        m["graphT"] = rhs_tiles(g, 8)
        in_maps.append(m)
    return in_maps, flags


def kernel(**inputs):
    in_maps, flags = _host_prep(inputs)
    key = tuple(sorted(flags.items()))
    if key not in _cache:
        _cache[key] = _build(flags)
    nc = _cache[key]
    res = run_bass_kernel_spmd(nc, in_maps, core_ids=list(range(N_CORES)))
    seq_out = np.concatenate([res.results[i]["seq_out"] for i in range(N_CORES)], axis=0)
    graph_out = np.concatenate([res.results[i]["graph_out"] for i in range(N_CORES)], axis=0)
    return (seq_out, graph_out)


# revision 27
# speedup vs baseline: 1.3767x; 1.1107x over previous
"""Trainium2 Bass kernel for nn_BimodalCrossAttentionBlock.

Math: seq-len-1 multihead cross attention => softmax over a single key is
identically 1, so MHA(x_q, x_kv) collapses to out_proj(v_proj(x_kv)) and the
two projections fold into one matrix Wc = out_w @ in_w[2D:] (Q/K projections
and num_heads are dead).  The block then is:
  graph_res = LN(graph + seq @ Wc_s2g.T + bc_s2g)     (gn1)
  seq_res   = LN(seq + graph @ Wc_g2s.T + bc_g2s)     (sn1)
  seq_out   = LN(seq_res + FFN_seq(seq_res))          (sn2)
  graph_out = LN(graph_res + FFN_gr(graph_res))       (gn2)

Sharding: pure data parallel over the batch dim (4096 rows/core x 8 cores);
weights replicated, no collectives.  Matmuls run in fp16 (fp32 PSUM accum),
skip paths / LayerNorm in fp32.  Host supplies the inputs in f16 in BOTH
batch-major and feature-major (pre-transposed) layouts, so phase A needs no
on-device input transposes.  Three on-device phases:
  A: attention + LN1 for both modalities (activation-stationary matmuls,
     batch-major outputs; LN rsqrt via Newton iteration on DVE); emits res in
     batch-major f16 (skip path) and feature-major f16 (FFN operand).
  B: seq FFN + LN2 (w1 prefetched during phase A; w2 streamed in ht-major
     chunks so the first matmuls start immediately)
  C: graph FFN + LN2
"""
import ml_dtypes
import numpy as np

import concourse.bass as bass
import concourse.bacc as bacc
import concourse.tile as tile
import concourse.mybir as mybir
from concourse.bass_utils import run_bass_kernel_spmd
from concourse.masks import make_identity

F8 = mybir.dt.float8e4
F16 = mybir.dt.float16
F32 = mybir.dt.float32
U32 = mybir.dt.uint32
AF = mybir.ActivationFunctionType
ALU = mybir.AluOpType
DRMODE = mybir.MatmulPerfMode.DoubleRow
E4NP = ml_dtypes.float8_e4m3

N_CORES = 8
B_FULL = 32768
D = 1024
HID = 4096
R = B_FULL // N_CORES
EPS = 1e-5
MAGIC = 0x5F3759DF

# Hidden tiles (of 32) computed through fp8 DoubleRow matmuls.  The fp8
# quantization noise scales as sqrt(NH8/32); NH8=8 simulates to a relative
# error of 1.35e-2 against the 2e-2 budget (f16-only is 4e-4).
NH8 = 8
NH16 = 32 - NH8

_cache = {}


def _ln_tail(nc, work, magic, x2, out_tile, lng_bc, lnb_bc):
    """LayerNorm of x2 [128, D] f32 -> out_tile; stats + rsqrt all on DVE."""
    stats = work.tile([128, 2, 6], F32, tag="lnstats")
    mv = work.tile([128, 2], F32, tag="lnmv")
    nc.vector.bn_stats(out=stats[:, 0, :], in_=x2[:, 0:512])
    nc.vector.bn_stats(out=stats[:, 1, :], in_=x2[:, 512:1024])
    nc.vector.bn_aggr(out=mv, in_=stats)
    v = work.tile([128, 1], F32, tag="lnv")
    nc.vector.tensor_scalar(out=v, in0=mv[:, 1:2], scalar1=EPS, scalar2=None,
                            op0=ALU.add)
    y = work.tile([128, 1], F32, tag="lny")
    t = work.tile([128, 1], F32, tag="lnt")
    nc.vector.tensor_scalar(out=y.bitcast(U32), in0=v.bitcast(U32), scalar1=1,
                            scalar2=None, op0=ALU.logical_shift_right)
    nc.vector.tensor_tensor(out=y.bitcast(U32), in0=magic, in1=y.bitcast(U32),
                            op=ALU.subtract)
    for _ in range(3):
        nc.vector.tensor_mul(out=t, in0=y, in1=y)
        nc.vector.tensor_mul(out=t, in0=t, in1=v)
        nc.vector.tensor_scalar(out=t, in0=t, scalar1=-0.5, scalar2=1.5,
                                op0=ALU.mult, op1=ALU.add)
        nc.vector.tensor_mul(out=y, in0=y, in1=t)
    if lng_bc is None and lnb_bc is None:
        nc.vector.tensor_scalar(out=out_tile, in0=x2, scalar1=mv[:, 0:1],
                                scalar2=y, op0=ALU.subtract, op1=ALU.mult)
    else:
        tmp = work.tile([128, 1024], F32, tag="lntmp")
        nc.vector.tensor_scalar(out=tmp, in0=x2, scalar1=mv[:, 0:1],
                                scalar2=y, op0=ALU.subtract, op1=ALU.mult)
        if lng_bc is not None:
            nc.vector.tensor_mul(out=tmp, in0=tmp, in1=lng_bc)
        if lnb_bc is not None:
            nc.vector.tensor_add(out=out_tile, in0=tmp, in1=lnb_bc)
        else:
            nc.vector.tensor_copy(out=out_tile, in_=tmp)


def _ln_dual_fast(nc, work, magic, x_s, x_g, res_s, res_g):
    """LayerNorm (no affine) of two [128, D] tiles; stats on DVE, the final
    normalize on the scalar engine (out = Identity(x * rstd - mu*rstd))."""
    stats = work.tile([128, 2, 2, 6], F32, tag="lnstats2")
    mv = work.tile([128, 2, 2], F32, tag="lnmv2")
    for i, x in enumerate((x_s, x_g)):
        nc.vector.bn_stats(out=stats[:, i, 0, :], in_=x[:, 0:512])
        nc.vector.bn_stats(out=stats[:, i, 1, :], in_=x[:, 512:1024])
        nc.vector.bn_aggr(out=mv[:, i, :], in_=stats[:, i, :, :])
    v = work.tile([128, 2], F32, tag="lnv2")
    nc.vector.tensor_scalar(out=v, in0=mv[:, :, 1], scalar1=EPS, scalar2=None,
                            op0=ALU.add)
    y = work.tile([128, 2], F32, tag="lny2")
    t = work.tile([128, 2], F32, tag="lnt2")
    nc.vector.tensor_scalar(out=y.bitcast(U32), in0=v.bitcast(U32), scalar1=1,
                            scalar2=None, op0=ALU.logical_shift_right)
    nc.vector.tensor_tensor(out=y.bitcast(U32), in0=magic, in1=y.bitcast(U32),
                            op=ALU.subtract)
    for _ in range(3):
        nc.vector.tensor_mul(out=t, in0=y, in1=y)
        nc.vector.tensor_mul(out=t, in0=t, in1=v)
        nc.vector.tensor_scalar(out=t, in0=t, scalar1=-0.5, scalar2=1.5,
                                op0=ALU.mult, op1=ALU.add)
        nc.vector.tensor_mul(out=y, in0=y, in1=t)
    nmy = work.tile([128, 2], F32, tag="lnnmy2")
    nc.vector.tensor_tensor(out=nmy, in0=mv[:, :, 0], in1=y, op=ALU.mult)
    nc.vector.tensor_scalar(out=nmy, in0=nmy, scalar1=-1.0, scalar2=None,
                            op0=ALU.mult)
    for i, (x, res) in enumerate(((x_s, res_s), (x_g, res_g))):
        nc.scalar.activation(out=res, in_=x, func=AF.Identity,
                             scale=y[:, i:i + 1], bias=nmy[:, i:i + 1])


def _bcast_param(nc, pool, dram_ap, n, tag):
    t = pool.tile([128, n], F32, tag=tag)
    src = bass.AP(tensor=dram_ap.tensor, offset=dram_ap.offset,
                  ap=[[0, 128]] + dram_ap.ap)
    nc.gpsimd.dma_start(out=t, in_=src)
    return t


def _build(flags):
    fl = lambda k: bool(flags.get(k, False))
    nc = bacc.Bacc("TRN2", target_bir_lowering=False, debug=False,
                   num_devices=N_CORES)

    seq16 = nc.declare_dram_parameter("seq16", [R, D], F16, isOutput=False)
    graph16 = nc.declare_dram_parameter("graph16", [R, D], F16, isOutput=False)
    seqT = nc.declare_dram_parameter("seqT", [128, 8, R], F16, isOutput=False)
    graphT = nc.declare_dram_parameter("graphT", [128, 8, R], F16, isOutput=False)
    wcs = nc.declare_dram_parameter("wcs", [128, 8, D], F16, isOutput=False)
    wcg = nc.declare_dram_parameter("wcg", [128, 8, D], F16, isOutput=False)
    w1s = nc.declare_dram_parameter("w1s", [128, 8, HID - NH8 * 128], F16, isOutput=False)
    w2s = nc.declare_dram_parameter("w2s", [128, NH16, D], F16, isOutput=False)
    w1g = nc.declare_dram_parameter("w1g", [128, 8, HID - NH8 * 128], F16, isOutput=False)
    w2g = nc.declare_dram_parameter("w2g", [128, NH16, D], F16, isOutput=False)
    w18s = nc.declare_dram_parameter("w18s", [128, 8, NH8 * 128], F8, isOutput=False)
    w28s = nc.declare_dram_parameter("w28s", [128, NH8, D], F8, isOutput=False)
    w18g = nc.declare_dram_parameter("w18g", [128, 8, NH8 * 128], F8, isOutput=False)
    w28g = nc.declare_dram_parameter("w28g", [128, NH8, D], F8, isOutput=False)
    opt = {}
    for nm, shape, dt in [("bcs", [1, D], F16), ("bcg", [1, D], F16),
                          ("b1s", [128, 32], F32), ("b1g", [128, 32], F32),
                          ("b2s", [1, D], F16), ("b2g", [1, D], F16),
                          ("sn1_g", [D], F32), ("sn1_b", [D], F32),
                          ("sn2_g", [D], F32), ("sn2_b", [D], F32),
                          ("gn1_g", [D], F32), ("gn1_b", [D], F32),
                          ("gn2_g", [D], F32), ("gn2_b", [D], F32)]:
        if fl(nm):
            opt[nm] = nc.declare_dram_parameter(nm, shape, dt, isOutput=False)
    seq_out = nc.declare_dram_parameter("seq_out", [R, D], F32, isOutput=True)
    graph_out = nc.declare_dram_parameter("graph_out", [R, D], F32, isOutput=True)

    NT = R // 128
    NB = R // 256

    with tile.TileContext(nc) as tc:
        with tc.tile_pool(name="dram", bufs=1, space="DRAM") as dram_pool, \
             tc.tile_pool(name="wpre", bufs=1) as wpre:
            sB_s = dram_pool.tile([R, D], F16)
            sB_g = dram_pool.tile([R, D], F16)
            sT_s = dram_pool.tile([128, 8, R], F16)
            sT_g = dram_pool.tile([128, 8, R], F16)

            # w1 buffers shared by phases B and C (re-filled for C); the
            # seq-FFN w1 prefetches during phase A in ht-major chunks.
            w18_sb = wpre.tile([128, 8, NH8 * 128], F8)
            nc.sync.dma_start(out=w18_sb, in_=w18s[:, :, :])
            w1_sb = wpre.tile([128, 8, HID - NH8 * 128], F16)
            for hs in range(6):
                nc.sync.dma_start(out=w1_sb[:, :, hs * 512:(hs + 1) * 512],
                                  in_=w1s[:, :, hs * 512:(hs + 1) * 512])

            # ---------------- Phase A: attention + LN1 ----------------
            with tc.tile_pool(name="singlesA", bufs=1) as singles, \
                 tc.tile_pool(name="workA", bufs=4) as work, \
                 tc.tile_pool(name="pstpA", bufs=3, space="PSUM") as pstp, \
                 tc.tile_pool(name="psmmA", bufs=5, space="PSUM") as psmm:

                wcs_sb = singles.tile([128, 8, D], F16)
                wcg_sb = singles.tile([128, 8, D], F16)
                for kt in range(8):
                    nc.sync.dma_start(out=wcs_sb[:, kt, :], in_=wcs[:, kt, :])
                    nc.sync.dma_start(out=wcg_sb[:, kt, :], in_=wcg[:, kt, :])
                ident16 = singles.tile([128, 128], F16)
                make_identity(nc, ident16)
                magic = singles.tile([128, 2], U32)
                nc.vector.memset(magic, MAGIC)
                ones16 = None
                opt_sb = {}
                if fl("bcs") or fl("bcg"):
                    ones16 = singles.tile([1, 128], F16)
                    nc.vector.memset(ones16, 1.0)
                    for nm in ("bcs", "bcg"):
                        if nm in opt:
                            opt_sb[nm] = singles.tile([1, D], F16, name=f"sb_{nm}")
                            nc.sync.dma_start(out=opt_sb[nm], in_=opt[nm][:, :])
                ln_bcs = {}
                for nm in ("sn1_g", "sn1_b", "gn1_g", "gn1_b"):
                    if nm in opt:
                        ln_bcs[nm] = _bcast_param(nc, singles, opt[nm].ap(), D, nm)

                for t in range(NT):
                    row = t * 128
                    S16 = work.tile([128, D], F16, tag="S16")
                    nc.sync.dma_start(out=S16, in_=seq16[row:row + 128, :])
                    G16 = work.tile([128, D], F16, tag="G16")
                    nc.sync.dma_start(out=G16, in_=graph16[row:row + 128, :])
                    ST = work.tile([128, 8, 128], F16, tag="ST")
                    nc.sync.dma_start(out=ST, in_=seqT[:, :, row:row + 128])
                    GT = work.tile([128, 8, 128], F16, tag="GT")
                    nc.sync.dma_start(out=GT, in_=graphT[:, :, row:row + 128])

                    x_s = work.tile([128, D], F16, tag="xs")
                    x_g = work.tile([128, D], F16, tag="xg")
                    # interleave the four attention psum tiles; evacuate each
                    # with its residual add as soon as its group completes so
                    # the next tile's matmuls aren't starved of PSUM.
                    for half in range(2):
                        nsl = slice(half * 512, (half + 1) * 512)
                        ga = psmm.tile([128, 512], F32, tag="attnps",
                                       name=f"ga{t}_{half}")
                        for kt in range(8):
                            nc.tensor.matmul(ga, lhsT=ST[:, kt, :],
                                             rhs=wcs_sb[:, kt, nsl],
                                             start=(kt == 0),
                                             stop=(kt == 7 and not fl("bcs")))
                        if fl("bcs"):
                            nc.tensor.matmul(ga, lhsT=ones16,
                                             rhs=opt_sb["bcs"][:, nsl],
                                             start=False, stop=True)
                        nc.vector.tensor_add(out=x_g[:, nsl], in0=G16[:, nsl],
                                             in1=ga)
                        sa = psmm.tile([128, 512], F32, tag="attnps",
                                       name=f"sa{t}_{half}")
                        for kt in range(8):
                            nc.tensor.matmul(sa, lhsT=GT[:, kt, :],
                                             rhs=wcg_sb[:, kt, nsl],
                                             start=(kt == 0),
                                             stop=(kt == 7 and not fl("bcg")))
                        if fl("bcg"):
                            nc.tensor.matmul(sa, lhsT=ones16,
                                             rhs=opt_sb["bcg"][:, nsl],
                                             start=False, stop=True)
                        nc.vector.tensor_add(out=x_s[:, nsl], in0=S16[:, nsl],
                                             in1=sa)

                    res16_s = work.tile([128, D], F16, tag="res16s")
                    res16_g = work.tile([128, D], F16, tag="res16g")
                    if ln_bcs:
                        for x, res16, g_nm, b_nm in (
                            (x_s, res16_s, "sn1_g", "sn1_b"),
                            (x_g, res16_g, "gn1_g", "gn1_b"),
                        ):
                            _ln_tail(nc, work, magic[:, 0:1], x, res16,
                                     ln_bcs.get(g_nm), ln_bcs.get(b_nm))
                    else:
                        _ln_dual_fast(nc, work, magic, x_s, x_g,
                                      res16_s, res16_g)

                    for which, res16, sB_d, sT_d in (
                        ("s", res16_s, sB_s, sT_s),
                        ("g", res16_g, sB_g, sT_g),
                    ):
                        rTt = work.tile([128, 8, 128], F16, tag=f"rTt{which}")
                        for grp in range(2):
                            # padded to a full PSUM bank to avoid bank-sharing
                            # serialization between transpose writes and the
                            # evacuation reads of the neighbouring buffer.
                            tpr = pstp.tile([128, 1024], F16, tag="tp",
                                            name=f"tpr{which}{t}_{grp}")
                            for j in range(4):
                                kt = grp * 4 + j
                                nc.tensor.transpose(tpr[:, j * 128:(j + 1) * 128],
                                                    res16[:, kt * 128:(kt + 1) * 128],
                                                    ident16)
                            nc.vector.tensor_copy(
                                out=rTt[:, grp * 4:(grp + 1) * 4, :].rearrange("p a b -> p (a b)"),
                                in_=tpr[:, 0:512])
                        nc.sync.dma_start(out=sT_d[:, :, row:row + 128], in_=rTt)
                        nc.sync.dma_start(out=sB_d[row:row + 128, :], in_=res16)

            # ---------------- Phases B/C: FFN + LN2 ----------------
            for ph, (w1_in, w18_in, w2_in, w28_in, sB_d, sT_d, out_d, b1_nm, b2_nm, g_nm, b_nm) in enumerate((
                (w1s, w18s, w2s, w28s, sB_s, sT_s, seq_out, "b1s", "b2s", "sn2_g", "sn2_b"),
                (w1g, w18g, w2g, w28g, sB_g, sT_g, graph_out, "b1g", "b2g", "gn2_g", "gn2_b"),
            )):
                with tc.tile_pool(name=f"singles{ph}", bufs=1) as singles, \
                     tc.tile_pool(name=f"work{ph}", bufs=3) as work, \
                     tc.tile_pool(name=f"hg{ph}", bufs=8) as hgpool, \
                     tc.tile_pool(name=f"psh{ph}", bufs=3, space="PSUM") as psh, \
                     tc.tile_pool(name=f"pso{ph}", bufs=4, space="PSUM") as pso:

                    # stage the first block's activations ahead of the bulk
                    # weight DMA so the phase's first matmuls aren't queued
                    # behind 8+ MB of weight traffic.
                    rT0 = work.tile([128, 8, 256], F16, tag="rT")
                    nc.sync.dma_start(out=rT0, in_=sT_d[:, :, 0:256])
                    rB0 = work.tile([128, 2, D], F16, tag="rB")
                    nc.sync.dma_start(
                        out=rB0,
                        in_=sB_d[0:256, :].rearrange("(s p) n -> p s n", p=128))
                    if ph != 0:
                        # refill the shared w1 buffers for the graph FFN;
                        # dependency tracking delays each chunk's DMA until
                        # phase B's reads of that region have completed.
                        nc.sync.dma_start(out=w18_sb, in_=w18_in[:, :, :])
                        for hs in range(6):
                            nc.sync.dma_start(
                                out=w1_sb[:, :, hs * 512:(hs + 1) * 512],
                                in_=w1_in[:, :, hs * 512:(hs + 1) * 512])
                    w28_sb = singles.tile([128, NH8, D], F8)
                    nc.sync.dma_start(out=w28_sb, in_=w28_in[:, :, :])
                    w2_sb = singles.tile([128, NH16, D], F16)
                    for ktg in range(6):
                        nc.sync.dma_start(out=w2_sb[:, ktg * 4:(ktg + 1) * 4, :],
                                          in_=w2_in[:, ktg * 4:(ktg + 1) * 4, :])
                    magic = singles.tile([128, 1], U32)
                    nc.vector.memset(magic, MAGIC)
                    b1_sb = None
                    if b1_nm in opt:
                        b1_sb = singles.tile([128, 32], F32)
                        nc.sync.dma_start(out=b1_sb, in_=opt[b1_nm][:, :])
                    ones16 = None
                    b2_sb = None
                    if b2_nm in opt:
                        ones16 = singles.tile([1, 128], F16)
                        nc.vector.memset(ones16, 1.0)
                        b2_sb = singles.tile([1, D], F16)
                        nc.sync.dma_start(out=b2_sb, in_=opt[b2_nm][:, :])
                    ln_g_bc = (_bcast_param(nc, singles, opt[g_nm].ap(), D, g_nm)
                               if g_nm in opt else None)
                    ln_b_bc = (_bcast_param(nc, singles, opt[b_nm].ap(), D, b_nm)
                               if b_nm in opt else None)

                    for blk in range(NB):
                        brow = blk * 256
                        if blk == 0:
                            rT, rB = rT0, rB0
                        else:
                            rT = work.tile([128, 8, 256], F16, tag="rT")
                            nc.sync.dma_start(out=rT, in_=sT_d[:, :, brow:brow + 256])
                            rB = work.tile([128, 2, D], F16, tag="rB")
                            nc.sync.dma_start(
                                out=rB,
                                in_=sB_d[brow:brow + 256, :].rearrange("(s p) n -> p s n", p=128))
                        rT8 = work.tile([128, 8, 256], F8, tag="rT8")
                        nc.vector.tensor_copy(out=rT8, in_=rT)
                        ops = [pso.tile([128, 512], F32, tag="ops", name=f"ops{blk}_{_h}")
                               for _h in range(4)]

                        # software-pipelined: mm2 for a hidden tile is emitted
                        # one iteration later, so its gelu output has been
                        # ready for a full cycle when the matmuls issue
                        # (otherwise the first mm2 stalls ~140 ns on the
                        # weight load).
                        def emit_mm2_f8(h8p, htp):
                            for bs in range(2):
                                for nh in range(2):
                                    nc.tensor.matmul(
                                        ops[bs * 2 + nh],
                                        lhsT=h8p[:, :, bs * 128:(bs + 1) * 128],
                                        rhs=w28_sb[:, 2 * htp:2 * htp + 2,
                                                   nh * 512:(nh + 1) * 512],
                                        start=(htp == 0), stop=False,
                                        perf_mode=DRMODE)

                        def emit_mm2_f16(hg_p, ht_p):
                            for bs in range(2):
                                for nh in range(2):
                                    nc.tensor.matmul(
                                        ops[bs * 2 + nh],
                                        lhsT=hg_p[:, bs * 128:(bs + 1) * 128],
                                        rhs=w2_sb[:, ht_p, nh * 512:(nh + 1) * 512],
                                        start=(NH8 == 0 and ht_p == 0),
                                        stop=(ht_p == NH16 - 1 and b2_sb is None))

                        pipe = None
                        # fp8 DoubleRow hidden tiles, processed in pairs
                        for htp in range(NH8 // 2):
                            h8p = hgpool.tile([128, 2, 256], F8, tag="h8p",
                                              name=f"h8p{blk}_{htp}")
                            for sub in range(2):
                                ht = htp * 2 + sub
                                # full-bank psum tile (2 KiB) so consecutive
                                # hps buffers never share a PSUM bank — a
                                # shared bank serializes the gelu read
                                # against the next mm1's writes.
                                hps = psh.tile([128, 512], F32, tag="hps",
                                               name=f"hps8{blk}_{ht}")
                                for j in range(4):
                                    nc.tensor.matmul(
                                        hps[:, 0:256],
                                        lhsT=w18_sb[:, 2 * j:2 * j + 2,
                                                    ht * 128:(ht + 1) * 128],
                                        rhs=rT8[:, 2 * j:2 * j + 2, :],
                                        start=(j == 0), stop=(j == 3),
                                        perf_mode=DRMODE)
                                if b1_sb is not None:
                                    nc.scalar.activation(
                                        out=h8p[:, sub, :], in_=hps[:, 0:256],
                                        func=AF.Gelu, bias=b1_sb[:, ht:ht + 1],
                                        scale=1.0, alpha=0.0)
                                else:
                                    nc.scalar.activation(
                                        out=h8p[:, sub, :], in_=hps[:, 0:256],
                                        func=AF.Gelu)
                            if pipe is not None:
                                pipe()
                            pipe = (lambda t=h8p, p=htp: emit_mm2_f8(t, p))
                        # f16 hidden tiles
                        for ht16 in range(NH16):
                            hps = psh.tile([128, 512], F32, tag="hps",
                                           name=f"hps16{blk}_{ht16}")
                            for kt in range(8):
                                nc.tensor.matmul(hps[:, 0:256],
                                                 lhsT=w1_sb[:, kt, ht16 * 128:(ht16 + 1) * 128],
                                                 rhs=rT[:, kt, :],
                                                 start=(kt == 0), stop=(kt == 7))
                            hg = hgpool.tile([128, 256], F16, tag="hg")
                            if b1_sb is not None:
                                nc.scalar.activation(out=hg, in_=hps[:, 0:256],
                                                     func=AF.Gelu,
                                                     bias=b1_sb[:, NH8 + ht16:NH8 + ht16 + 1],
                                                     scale=1.0, alpha=0.0)
                            else:
                                nc.scalar.activation(out=hg, in_=hps[:, 0:256],
                                                     func=AF.Gelu)
                            if pipe is not None:
                                pipe()
                            pipe = (lambda t=hg, p=ht16: emit_mm2_f16(t, p))
                        pipe()
                        if b2_sb is not None:
                            for bs in range(2):
                                for nh in range(2):
                                    nc.tensor.matmul(ops[bs * 2 + nh], lhsT=ones16,
                                                     rhs=b2_sb[:, nh * 512:(nh + 1) * 512],
                                                     start=False, stop=True)
                        for bs in range(2):
                            x2 = work.tile([128, D], F32, tag="x2")
                            nc.vector.tensor_add(out=x2[:, 0:512], in0=rB[:, bs, 0:512],
                                                 in1=ops[bs * 2 + 0])
                            nc.vector.tensor_add(out=x2[:, 512:1024],
                                                 in0=rB[:, bs, 512:1024],
                                                 in1=ops[bs * 2 + 1])
                            ot = work.tile([128, D], F32, tag="ot")
                            _ln_tail(nc, work, magic, x2, ot, ln_g_bc, ln_b_bc)
                            nc.sync.dma_start(
                                out=out_d[brow + bs * 128:brow + bs * 128 + 128, :],
                                in_=ot)

    nc.compile()
    return nc


def _host_prep(inputs):
    f = lambda k: np.asarray(inputs[k])
    flags = {}

    def fold(pfx):
        in_w = f(f"{pfx}_in_w").astype(np.float64)
        in_b = f(f"{pfx}_in_b").astype(np.float64)
        out_w = f(f"{pfx}_out_w").astype(np.float64)
        out_b = f(f"{pfx}_out_b").astype(np.float64)
        Wc = out_w @ in_w[2 * D:]
        bc = in_b[2 * D:] @ out_w.T + out_b
        return Wc, bc

    Wcs, bcs = fold("s2g")
    Wcg, bcg = fold("g2s")

    def base_tiles(W, kt):  # W [n, d_in] -> [128, kt, n] tiles of W.T
        return np.ascontiguousarray(W.T.reshape(kt, 128, -1).transpose(1, 0, 2))

    def rhs_tiles(W, kt):
        return base_tiles(W, kt).astype(np.float16)

    wm = {"wcs": rhs_tiles(Wcs, 8), "wcg": rhs_tiles(Wcg, 8)}
    for sfx, w1k, w2k in (("s", "seq_w1", "seq_w2"), ("g", "gr_w1", "gr_w2")):
        w1t = base_tiles(f(w1k), 8)          # [128, 8, HID]
        w2t = base_tiles(f(w2k), 32)         # [128, 32, D]
        wm[f"w18{sfx}"] = np.ascontiguousarray(w1t[:, :, :NH8 * 128]).astype(E4NP)
        wm[f"w1{sfx}"] = np.ascontiguousarray(w1t[:, :, NH8 * 128:]).astype(np.float16)
        wm[f"w28{sfx}"] = np.ascontiguousarray(w2t[:, :NH8, :]).astype(E4NP)
        wm[f"w2{sfx}"] = np.ascontiguousarray(w2t[:, NH8:, :]).astype(np.float16)
    if np.any(bcs != 0):
        flags["bcs"] = True
        wm["bcs"] = bcs.astype(np.float16).reshape(1, D)
    if np.any(bcg != 0):
        flags["bcg"] = True
        wm["bcg"] = bcg.astype(np.float16).reshape(1, D)
    for nm, key in (("b1s", "seq_b1"), ("b1g", "gr_b1")):
        v = f(key)
        if np.any(v != 0):
            flags[nm] = True
            wm[nm] = np.ascontiguousarray(v.reshape(32, 128).T).astype(np.float32)
    for nm, key in (("b2s", "seq_b2"), ("b2g", "gr_b2")):
        v = f(key)
        if np.any(v != 0):
            flags[nm] = True
            wm[nm] = v.astype(np.float16).reshape(1, D)
    for nm in ("sn1", "sn2", "gn1", "gn2"):
        g = f(f"{nm}_g"); b = f(f"{nm}_b")
        if np.any(g != 1):
            flags[f"{nm}_g"] = True
            wm[f"{nm}_g"] = g.astype(np.float32)
        if np.any(b != 0):
            flags[f"{nm}_b"] = True
            wm[f"{nm}_b"] = b.astype(np.float32)

    seq = np.asarray(f("seq_emb"), dtype=np.float32)
    graph = np.asarray(f("graph_emb"), dtype=np.float32)
    seq16_full = seq.astype(np.float16)
    graph16_full = graph.astype(np.float16)
    in_maps = []
    for i in range(N_CORES):
        m = dict(wm)
        s = seq16_full[i * R:(i + 1) * R]
        g = graph16_full[i * R:(i + 1) * R]
        m["seq16"] = np.ascontiguousarray(s)
        m["graph16"] = np.ascontiguousarray(g)
        m["seqT"] = rhs_tiles(s, 8)
        m["graphT"] = rhs_tiles(g, 8)
        in_maps.append(m)
    return in_maps, flags


def kernel(**inputs):
    in_maps, flags = _host_prep(inputs)
    key = tuple(sorted(flags.items()))
    if key not in _cache:
        _cache[key] = _build(flags)
    nc = _cache[key]
    res = run_bass_kernel_spmd(nc, in_maps, core_ids=list(range(N_CORES)))
    seq_out = np.concatenate([res.results[i]["seq_out"] for i in range(N_CORES)], axis=0)
    graph_out = np.concatenate([res.results[i]["graph_out"] for i in range(N_CORES)], axis=0)
    return (seq_out, graph_out)


# revision 29
# speedup vs baseline: 1.4145x; 1.0275x over previous
"""Trainium2 Bass kernel for nn_BimodalCrossAttentionBlock.

Math: seq-len-1 multihead cross attention => softmax over a single key is
identically 1, so MHA(x_q, x_kv) collapses to out_proj(v_proj(x_kv)) and the
two projections fold into one matrix Wc = out_w @ in_w[2D:] (Q/K projections
and num_heads are dead).  The block then is:
  graph_res = LN(graph + seq @ Wc_s2g.T + bc_s2g)     (gn1)
  seq_res   = LN(seq + graph @ Wc_g2s.T + bc_g2s)     (sn1)
  seq_out   = LN(seq_res + FFN_seq(seq_res))          (sn2)
  graph_out = LN(graph_res + FFN_gr(graph_res))       (gn2)

Sharding: pure data parallel over the batch dim (4096 rows/core x 8 cores);
weights replicated, no collectives.  Matmuls run in fp16 (fp32 PSUM accum),
skip paths / LayerNorm in fp32.  Host supplies the inputs in f16 in BOTH
batch-major and feature-major (pre-transposed) layouts, so phase A needs no
on-device input transposes.  Three on-device phases:
  A: attention + LN1 for both modalities (activation-stationary matmuls,
     batch-major outputs; LN rsqrt via Newton iteration on DVE); emits res in
     batch-major f16 (skip path) and feature-major f16 (FFN operand).
  B: seq FFN + LN2 (w1 prefetched during phase A; w2 streamed in ht-major
     chunks so the first matmuls start immediately)
  C: graph FFN + LN2
"""
import ml_dtypes
import numpy as np

import concourse.bass as bass
import concourse.bacc as bacc
import concourse.tile as tile
import concourse.mybir as mybir
from concourse.bass_utils import run_bass_kernel_spmd
from concourse.masks import make_identity

F8 = mybir.dt.float8e4
F16 = mybir.dt.float16
F32 = mybir.dt.float32
U32 = mybir.dt.uint32
AF = mybir.ActivationFunctionType
ALU = mybir.AluOpType
DRMODE = mybir.MatmulPerfMode.DoubleRow
E4NP = ml_dtypes.float8_e4m3

N_CORES = 8
B_FULL = 32768
D = 1024
HID = 4096
R = B_FULL // N_CORES
EPS = 1e-5
MAGIC = 0x5F3759DF

# Hidden tiles (of 32) computed through fp8 DoubleRow matmuls.  The fp8
# quantization noise scales as sqrt(NH8/32); NH8=8 measured 1.44e-2 on HW
# against the 2e-2 budget (f16-only is 4e-4).
NH8 = 12
NH16 = 32 - NH8

_cache = {}


def _ln_tail(nc, work, magic, x2, out_tile, lng_bc, lnb_bc):
    """LayerNorm of x2 [128, D] f32 -> out_tile; stats + rsqrt all on DVE."""
    stats = work.tile([128, 2, 6], F32, tag="lnstats")
    mv = work.tile([128, 2], F32, tag="lnmv")
    nc.vector.bn_stats(out=stats[:, 0, :], in_=x2[:, 0:512])
    nc.vector.bn_stats(out=stats[:, 1, :], in_=x2[:, 512:1024])
    nc.vector.bn_aggr(out=mv, in_=stats)
    v = work.tile([128, 1], F32, tag="lnv")
    nc.vector.tensor_scalar(out=v, in0=mv[:, 1:2], scalar1=EPS, scalar2=None,
                            op0=ALU.add)
    y = work.tile([128, 1], F32, tag="lny")
    t = work.tile([128, 1], F32, tag="lnt")
    nc.vector.tensor_scalar(out=y.bitcast(U32), in0=v.bitcast(U32), scalar1=1,
                            scalar2=None, op0=ALU.logical_shift_right)
    nc.vector.tensor_tensor(out=y.bitcast(U32), in0=magic, in1=y.bitcast(U32),
                            op=ALU.subtract)
    for _ in range(3):
        nc.vector.tensor_mul(out=t, in0=y, in1=y)
        nc.vector.tensor_mul(out=t, in0=t, in1=v)
        nc.vector.tensor_scalar(out=t, in0=t, scalar1=-0.5, scalar2=1.5,
                                op0=ALU.mult, op1=ALU.add)
        nc.vector.tensor_mul(out=y, in0=y, in1=t)
    if lng_bc is None and lnb_bc is None:
        nc.vector.tensor_scalar(out=out_tile, in0=x2, scalar1=mv[:, 0:1],
                                scalar2=y, op0=ALU.subtract, op1=ALU.mult)
    else:
        tmp = work.tile([128, 1024], F32, tag="lntmp")
        nc.vector.tensor_scalar(out=tmp, in0=x2, scalar1=mv[:, 0:1],
                                scalar2=y, op0=ALU.subtract, op1=ALU.mult)
        if lng_bc is not None:
            nc.vector.tensor_mul(out=tmp, in0=tmp, in1=lng_bc)
        if lnb_bc is not None:
            nc.vector.tensor_add(out=out_tile, in0=tmp, in1=lnb_bc)
        else:
            nc.vector.tensor_copy(out=out_tile, in_=tmp)


def _ln_dual_fast(nc, work, magic, x_s, x_g, res_s, res_g):
    """LayerNorm (no affine) of two [128, D] tiles; stats on DVE, the final
    normalize on the scalar engine (out = Identity(x * rstd - mu*rstd))."""
    stats = work.tile([128, 2, 2, 6], F32, tag="lnstats2")
    mv = work.tile([128, 2, 2], F32, tag="lnmv2")
    for i, x in enumerate((x_s, x_g)):
        nc.vector.bn_stats(out=stats[:, i, 0, :], in_=x[:, 0:512])
        nc.vector.bn_stats(out=stats[:, i, 1, :], in_=x[:, 512:1024])
        nc.vector.bn_aggr(out=mv[:, i, :], in_=stats[:, i, :, :])
    v = work.tile([128, 2], F32, tag="lnv2")
    nc.vector.tensor_scalar(out=v, in0=mv[:, :, 1], scalar1=EPS, scalar2=None,
                            op0=ALU.add)
    y = work.tile([128, 2], F32, tag="lny2")
    t = work.tile([128, 2], F32, tag="lnt2")
    nc.vector.tensor_scalar(out=y.bitcast(U32), in0=v.bitcast(U32), scalar1=1,
                            scalar2=None, op0=ALU.logical_shift_right)
    nc.vector.tensor_tensor(out=y.bitcast(U32), in0=magic, in1=y.bitcast(U32),
                            op=ALU.subtract)
    for _ in range(3):
        nc.vector.tensor_mul(out=t, in0=y, in1=y)
        nc.vector.tensor_mul(out=t, in0=t, in1=v)
        nc.vector.tensor_scalar(out=t, in0=t, scalar1=-0.5, scalar2=1.5,
                                op0=ALU.mult, op1=ALU.add)
        nc.vector.tensor_mul(out=y, in0=y, in1=t)
    nmy = work.tile([128, 2], F32, tag="lnnmy2")
    nc.vector.tensor_tensor(out=nmy, in0=mv[:, :, 0], in1=y, op=ALU.mult)
    nc.vector.tensor_scalar(out=nmy, in0=nmy, scalar1=-1.0, scalar2=None,
                            op0=ALU.mult)
    for i, (x, res) in enumerate(((x_s, res_s), (x_g, res_g))):
        nc.scalar.activation(out=res, in_=x, func=AF.Identity,
                             scale=y[:, i:i + 1], bias=nmy[:, i:i + 1])


def _bcast_param(nc, pool, dram_ap, n, tag):
    t = pool.tile([128, n], F32, tag=tag)
    src = bass.AP(tensor=dram_ap.tensor, offset=dram_ap.offset,
                  ap=[[0, 128]] + dram_ap.ap)
    nc.gpsimd.dma_start(out=t, in_=src)
    return t


def _build(flags):
    fl = lambda k: bool(flags.get(k, False))
    nc = bacc.Bacc("TRN2", target_bir_lowering=False, debug=False,
                   num_devices=N_CORES)

    seq16 = nc.declare_dram_parameter("seq16", [R, D], F16, isOutput=False)
    graph16 = nc.declare_dram_parameter("graph16", [R, D], F16, isOutput=False)
    seqT = nc.declare_dram_parameter("seqT", [128, 8, R], F16, isOutput=False)
    graphT = nc.declare_dram_parameter("graphT", [128, 8, R], F16, isOutput=False)
    wcs = nc.declare_dram_parameter("wcs", [128, 8, D], F16, isOutput=False)
    wcg = nc.declare_dram_parameter("wcg", [128, 8, D], F16, isOutput=False)
    w1s = nc.declare_dram_parameter("w1s", [128, 8, HID - NH8 * 128], F16, isOutput=False)
    w2s = nc.declare_dram_parameter("w2s", [128, NH16, D], F16, isOutput=False)
    w1g = nc.declare_dram_parameter("w1g", [128, 8, HID - NH8 * 128], F16, isOutput=False)
    w2g = nc.declare_dram_parameter("w2g", [128, NH16, D], F16, isOutput=False)
    w18s = nc.declare_dram_parameter("w18s", [128, 8, NH8 * 128], F8, isOutput=False)
    w28s = nc.declare_dram_parameter("w28s", [128, NH8, D], F8, isOutput=False)
    w18g = nc.declare_dram_parameter("w18g", [128, 8, NH8 * 128], F8, isOutput=False)
    w28g = nc.declare_dram_parameter("w28g", [128, NH8, D], F8, isOutput=False)
    opt = {}
    for nm, shape, dt in [("bcs", [1, D], F16), ("bcg", [1, D], F16),
                          ("b1s", [128, 32], F32), ("b1g", [128, 32], F32),
                          ("b2s", [1, D], F16), ("b2g", [1, D], F16),
                          ("sn1_g", [D], F32), ("sn1_b", [D], F32),
                          ("sn2_g", [D], F32), ("sn2_b", [D], F32),
                          ("gn1_g", [D], F32), ("gn1_b", [D], F32),
                          ("gn2_g", [D], F32), ("gn2_b", [D], F32)]:
        if fl(nm):
            opt[nm] = nc.declare_dram_parameter(nm, shape, dt, isOutput=False)
    seq_out = nc.declare_dram_parameter("seq_out", [R, D], F32, isOutput=True)
    graph_out = nc.declare_dram_parameter("graph_out", [R, D], F32, isOutput=True)

    NT = R // 128
    NB = R // 256

    with tile.TileContext(nc) as tc:
        with tc.tile_pool(name="dram", bufs=1, space="DRAM") as dram_pool, \
             tc.tile_pool(name="wpre", bufs=1) as wpre:
            sB_s = dram_pool.tile([R, D], F16)
            sB_g = dram_pool.tile([R, D], F16)
            sT_s = dram_pool.tile([128, 8, R], F16)
            sT_g = dram_pool.tile([128, 8, R], F16)

            # w1 buffers shared by phases B and C (re-filled for C); the
            # seq-FFN w1 prefetches during phase A in ht-major chunks.
            w18_sb = wpre.tile([128, 8, NH8 * 128], F8)
            nc.sync.dma_start(out=w18_sb, in_=w18s[:, :, :])
            w1_sb = wpre.tile([128, 8, HID - NH8 * 128], F16)
            for hs in range((HID - NH8 * 128) // 512):
                nc.sync.dma_start(out=w1_sb[:, :, hs * 512:(hs + 1) * 512],
                                  in_=w1s[:, :, hs * 512:(hs + 1) * 512])

            # ---------------- Phase A: attention + LN1 ----------------
            with tc.tile_pool(name="singlesA", bufs=1) as singles, \
                 tc.tile_pool(name="workA", bufs=4) as work, \
                 tc.tile_pool(name="pstpA", bufs=3, space="PSUM") as pstp, \
                 tc.tile_pool(name="psmmA", bufs=5, space="PSUM") as psmm:

                wcs_sb = singles.tile([128, 8, D], F16)
                wcg_sb = singles.tile([128, 8, D], F16)
                for kt in range(8):
                    nc.sync.dma_start(out=wcs_sb[:, kt, :], in_=wcs[:, kt, :])
                    nc.sync.dma_start(out=wcg_sb[:, kt, :], in_=wcg[:, kt, :])
                ident16 = singles.tile([128, 128], F16)
                make_identity(nc, ident16)
                magic = singles.tile([128, 2], U32)
                nc.vector.memset(magic, MAGIC)
                ones16 = None
                opt_sb = {}
                if fl("bcs") or fl("bcg"):
                    ones16 = singles.tile([1, 128], F16)
                    nc.vector.memset(ones16, 1.0)
                    for nm in ("bcs", "bcg"):
                        if nm in opt:
                            opt_sb[nm] = singles.tile([1, D], F16, name=f"sb_{nm}")
                            nc.sync.dma_start(out=opt_sb[nm], in_=opt[nm][:, :])
                ln_bcs = {}
                for nm in ("sn1_g", "sn1_b", "gn1_g", "gn1_b"):
                    if nm in opt:
                        ln_bcs[nm] = _bcast_param(nc, singles, opt[nm].ap(), D, nm)

                for t in range(NT):
                    row = t * 128
                    S16 = work.tile([128, D], F16, tag="S16")
                    nc.sync.dma_start(out=S16, in_=seq16[row:row + 128, :])
                    G16 = work.tile([128, D], F16, tag="G16")
                    nc.sync.dma_start(out=G16, in_=graph16[row:row + 128, :])
                    ST = work.tile([128, 8, 128], F16, tag="ST")
                    nc.sync.dma_start(out=ST, in_=seqT[:, :, row:row + 128])
                    GT = work.tile([128, 8, 128], F16, tag="GT")
                    nc.sync.dma_start(out=GT, in_=graphT[:, :, row:row + 128])

                    x_s = work.tile([128, D], F16, tag="xs")
                    x_g = work.tile([128, D], F16, tag="xg")
                    # interleave the four attention psum tiles; evacuate each
                    # with its residual add as soon as its group completes so
                    # the next tile's matmuls aren't starved of PSUM.
                    for half in range(2):
                        nsl = slice(half * 512, (half + 1) * 512)
                        ga = psmm.tile([128, 512], F32, tag="attnps",
                                       name=f"ga{t}_{half}")
                        for kt in range(8):
                            nc.tensor.matmul(ga, lhsT=ST[:, kt, :],
                                             rhs=wcs_sb[:, kt, nsl],
                                             start=(kt == 0),
                                             stop=(kt == 7 and not fl("bcs")))
                        if fl("bcs"):
                            nc.tensor.matmul(ga, lhsT=ones16,
                                             rhs=opt_sb["bcs"][:, nsl],
                                             start=False, stop=True)
                        nc.vector.tensor_add(out=x_g[:, nsl], in0=G16[:, nsl],
                                             in1=ga)
                        sa = psmm.tile([128, 512], F32, tag="attnps",
                                       name=f"sa{t}_{half}")
                        for kt in range(8):
                            nc.tensor.matmul(sa, lhsT=GT[:, kt, :],
                                             rhs=wcg_sb[:, kt, nsl],
                                             start=(kt == 0),
                                             stop=(kt == 7 and not fl("bcg")))
                        if fl("bcg"):
                            nc.tensor.matmul(sa, lhsT=ones16,
                                             rhs=opt_sb["bcg"][:, nsl],
                                             start=False, stop=True)
                        nc.vector.tensor_add(out=x_s[:, nsl], in0=S16[:, nsl],
                                             in1=sa)

                    res16_s = work.tile([128, D], F16, tag="res16s")
                    res16_g = work.tile([128, D], F16, tag="res16g")
                    if ln_bcs:
                        for x, res16, g_nm, b_nm in (
                            (x_s, res16_s, "sn1_g", "sn1_b"),
                            (x_g, res16_g, "gn1_g", "gn1_b"),
                        ):
                            _ln_tail(nc, work, magic[:, 0:1], x, res16,
                                     ln_bcs.get(g_nm), ln_bcs.get(b_nm))
                    else:
                        _ln_dual_fast(nc, work, magic, x_s, x_g,
                                      res16_s, res16_g)

                    for which, res16, sB_d, sT_d in (
                        ("s", res16_s, sB_s, sT_s),
                        ("g", res16_g, sB_g, sT_g),
                    ):
                        rTt = work.tile([128, 8, 128], F16, tag=f"rTt{which}")
                        for grp in range(2):
                            # padded to a full PSUM bank to avoid bank-sharing
                            # serialization between transpose writes and the
                            # evacuation reads of the neighbouring buffer.
                            tpr = pstp.tile([128, 1024], F16, tag="tp",
                                            name=f"tpr{which}{t}_{grp}")
                            for j in range(4):
                                kt = grp * 4 + j
                                nc.tensor.transpose(tpr[:, j * 128:(j + 1) * 128],
                                                    res16[:, kt * 128:(kt + 1) * 128],
                                                    ident16)
                            nc.vector.tensor_copy(
                                out=rTt[:, grp * 4:(grp + 1) * 4, :].rearrange("p a b -> p (a b)"),
                                in_=tpr[:, 0:512])
                        nc.sync.dma_start(out=sT_d[:, :, row:row + 128], in_=rTt)
                        nc.sync.dma_start(out=sB_d[row:row + 128, :], in_=res16)

            # ---------------- Phases B/C: FFN + LN2 ----------------
            for ph, (w1_in, w18_in, w2_in, w28_in, sB_d, sT_d, out_d, b1_nm, b2_nm, g_nm, b_nm) in enumerate((
                (w1s, w18s, w2s, w28s, sB_s, sT_s, seq_out, "b1s", "b2s", "sn2_g", "sn2_b"),
                (w1g, w18g, w2g, w28g, sB_g, sT_g, graph_out, "b1g", "b2g", "gn2_g", "gn2_b"),
            )):
                with tc.tile_pool(name=f"singles{ph}", bufs=1) as singles, \
                     tc.tile_pool(name=f"work{ph}", bufs=3) as work, \
                     tc.tile_pool(name=f"hg{ph}", bufs=8) as hgpool, \
                     tc.tile_pool(name=f"psh{ph}", bufs=3, space="PSUM") as psh, \
                     tc.tile_pool(name=f"pso{ph}", bufs=4, space="PSUM") as pso:

                    # stage the first block's activations ahead of the bulk
                    # weight DMA so the phase's first matmuls aren't queued
                    # behind 8+ MB of weight traffic.
                    rT0 = work.tile([128, 8, 256], F16, tag="rT")
                    nc.sync.dma_start(out=rT0, in_=sT_d[:, :, 0:256])
                    rB0 = work.tile([128, 2, D], F16, tag="rB")
                    nc.sync.dma_start(
                        out=rB0,
                        in_=sB_d[0:256, :].rearrange("(s p) n -> p s n", p=128))
                    if ph != 0:
                        # refill the shared w1 buffers for the graph FFN;
                        # dependency tracking delays each chunk's DMA until
                        # phase B's reads of that region have completed.
                        nc.sync.dma_start(out=w18_sb, in_=w18_in[:, :, :])
                        for hs in range((HID - NH8 * 128) // 512):
                            nc.sync.dma_start(
                                out=w1_sb[:, :, hs * 512:(hs + 1) * 512],
                                in_=w1_in[:, :, hs * 512:(hs + 1) * 512])
                    w28_sb = singles.tile([128, NH8, D], F8)
                    nc.sync.dma_start(out=w28_sb, in_=w28_in[:, :, :])
                    w2_sb = singles.tile([128, NH16, D], F16)
                    for ktg in range(NH16 // 4):
                        nc.sync.dma_start(out=w2_sb[:, ktg * 4:(ktg + 1) * 4, :],
                                          in_=w2_in[:, ktg * 4:(ktg + 1) * 4, :])
                    magic = singles.tile([128, 1], U32)
                    nc.vector.memset(magic, MAGIC)
                    b1_sb = None
                    if b1_nm in opt:
                        b1_sb = singles.tile([128, 32], F32)
                        nc.sync.dma_start(out=b1_sb, in_=opt[b1_nm][:, :])
                    ones16 = None
                    b2_sb = None
                    if b2_nm in opt:
                        ones16 = singles.tile([1, 128], F16)
                        nc.vector.memset(ones16, 1.0)
                        b2_sb = singles.tile([1, D], F16)
                        nc.sync.dma_start(out=b2_sb, in_=opt[b2_nm][:, :])
                    ln_g_bc = (_bcast_param(nc, singles, opt[g_nm].ap(), D, g_nm)
                               if g_nm in opt else None)
                    ln_b_bc = (_bcast_param(nc, singles, opt[b_nm].ap(), D, b_nm)
                               if b_nm in opt else None)

                    for blk in range(NB):
                        brow = blk * 256
                        if blk == 0:
                            rT, rB = rT0, rB0
                        else:
                            rT = work.tile([128, 8, 256], F16, tag="rT")
                            nc.sync.dma_start(out=rT, in_=sT_d[:, :, brow:brow + 256])
                            rB = work.tile([128, 2, D], F16, tag="rB")
                            nc.sync.dma_start(
                                out=rB,
                                in_=sB_d[brow:brow + 256, :].rearrange("(s p) n -> p s n", p=128))
                        rT8 = work.tile([128, 8, 256], F8, tag="rT8")
                        nc.vector.tensor_copy(out=rT8, in_=rT)
                        ops = [pso.tile([128, 512], F32, tag="ops", name=f"ops{blk}_{_h}")
                               for _h in range(4)]

                        # software-pipelined: mm2 for a hidden tile is emitted
                        # one iteration later, so its gelu output has been
                        # ready for a full cycle when the matmuls issue
                        # (otherwise the first mm2 stalls ~140 ns on the
                        # weight load).
                        def emit_mm2_f8(h8p, htp):
                            for bs in range(2):
                                for nh in range(2):
                                    nc.tensor.matmul(
                                        ops[bs * 2 + nh],
                                        lhsT=h8p[:, :, bs * 128:(bs + 1) * 128],
                                        rhs=w28_sb[:, 2 * htp:2 * htp + 2,
                                                   nh * 512:(nh + 1) * 512],
                                        start=(htp == 0), stop=False,
                                        perf_mode=DRMODE)

                        def emit_mm2_f16(hg_p, ht_p):
                            for bs in range(2):
                                for nh in range(2):
                                    nc.tensor.matmul(
                                        ops[bs * 2 + nh],
                                        lhsT=hg_p[:, bs * 128:(bs + 1) * 128],
                                        rhs=w2_sb[:, ht_p, nh * 512:(nh + 1) * 512],
                                        start=(NH8 == 0 and ht_p == 0),
                                        stop=(ht_p == NH16 - 1 and b2_sb is None))

                        pipe = None
                        # fp8 DoubleRow hidden tiles, processed in pairs
                        for htp in range(NH8 // 2):
                            h8p = hgpool.tile([128, 2, 256], F8, tag="h8p",
                                              name=f"h8p{blk}_{htp}")
                            for sub in range(2):
                                ht = htp * 2 + sub
                                # full-bank psum tile (2 KiB) so consecutive
                                # hps buffers never share a PSUM bank — a
                                # shared bank serializes the gelu read
                                # against the next mm1's writes.
                                hps = psh.tile([128, 512], F32, tag="hps",
                                               name=f"hps8{blk}_{ht}")
                                for j in range(4):
                                    nc.tensor.matmul(
                                        hps[:, 0:256],
                                        lhsT=w18_sb[:, 2 * j:2 * j + 2,
                                                    ht * 128:(ht + 1) * 128],
                                        rhs=rT8[:, 2 * j:2 * j + 2, :],
                                        start=(j == 0), stop=(j == 3),
                                        perf_mode=DRMODE)
                                if b1_sb is not None:
                                    nc.scalar.activation(
                                        out=h8p[:, sub, :], in_=hps[:, 0:256],
                                        func=AF.Gelu, bias=b1_sb[:, ht:ht + 1],
                                        scale=1.0, alpha=0.0)
                                else:
                                    nc.scalar.activation(
                                        out=h8p[:, sub, :], in_=hps[:, 0:256],
                                        func=AF.Gelu)
                            if pipe is not None:
                                pipe()
                            pipe = (lambda t=h8p, p=htp: emit_mm2_f8(t, p))
                        # f16 hidden tiles
                        for ht16 in range(NH16):
                            hps = psh.tile([128, 512], F32, tag="hps",
                                           name=f"hps16{blk}_{ht16}")
                            for kt in range(8):
                                nc.tensor.matmul(hps[:, 0:256],
                                                 lhsT=w1_sb[:, kt, ht16 * 128:(ht16 + 1) * 128],
                                                 rhs=rT[:, kt, :],
                                                 start=(kt == 0), stop=(kt == 7))
                            hg = hgpool.tile([128, 256], F16, tag="hg")
                            if b1_sb is not None:
                                nc.scalar.activation(out=hg, in_=hps[:, 0:256],
                                                     func=AF.Gelu,
                                                     bias=b1_sb[:, NH8 + ht16:NH8 + ht16 + 1],
                                                     scale=1.0, alpha=0.0)
                            else:
                                nc.scalar.activation(out=hg, in_=hps[:, 0:256],
                                                     func=AF.Gelu)
                            if pipe is not None:
                                pipe()
                            pipe = (lambda t=hg, p=ht16: emit_mm2_f16(t, p))
                        pipe()
                        if b2_sb is not None:
                            for bs in range(2):
                                for nh in range(2):
                                    nc.tensor.matmul(ops[bs * 2 + nh], lhsT=ones16,
                                                     rhs=b2_sb[:, nh * 512:(nh + 1) * 512],
                                                     start=False, stop=True)
                        for bs in range(2):
                            x2 = work.tile([128, D], F32, tag="x2")
                            nc.vector.tensor_add(out=x2[:, 0:512], in0=rB[:, bs, 0:512],
                                                 in1=ops[bs * 2 + 0])
                            nc.vector.tensor_add(out=x2[:, 512:1024],
                                                 in0=rB[:, bs, 512:1024],
                                                 in1=ops[bs * 2 + 1])
                            ot = work.tile([128, D], F32, tag="ot")
                            _ln_tail(nc, work, magic, x2, ot, ln_g_bc, ln_b_bc)
                            nc.sync.dma_start(
                                out=out_d[brow + bs * 128:brow + bs * 128 + 128, :],
                                in_=ot)

    nc.compile()
    return nc


def _host_prep(inputs):
    f = lambda k: np.asarray(inputs[k])
    flags = {}

    def fold(pfx):
        in_w = f(f"{pfx}_in_w").astype(np.float64)
        in_b = f(f"{pfx}_in_b").astype(np.float64)
        out_w = f(f"{pfx}_out_w").astype(np.float64)
        out_b = f(f"{pfx}_out_b").astype(np.float64)
        Wc = out_w @ in_w[2 * D:]
        bc = in_b[2 * D:] @ out_w.T + out_b
        return Wc, bc

    Wcs, bcs = fold("s2g")
    Wcg, bcg = fold("g2s")

    def base_tiles(W, kt):  # W [n, d_in] -> [128, kt, n] tiles of W.T
        return np.ascontiguousarray(W.T.reshape(kt, 128, -1).transpose(1, 0, 2))

    def rhs_tiles(W, kt):
        return base_tiles(W, kt).astype(np.float16)

    wm = {"wcs": rhs_tiles(Wcs, 8), "wcg": rhs_tiles(Wcg, 8)}
    for sfx, w1k, w2k in (("s", "seq_w1", "seq_w2"), ("g", "gr_w1", "gr_w2")):
        w1t = base_tiles(f(w1k), 8)          # [128, 8, HID]
        w2t = base_tiles(f(w2k), 32)         # [128, 32, D]
        wm[f"w18{sfx}"] = np.ascontiguousarray(w1t[:, :, :NH8 * 128]).astype(E4NP)
        wm[f"w1{sfx}"] = np.ascontiguousarray(w1t[:, :, NH8 * 128:]).astype(np.float16)
        wm[f"w28{sfx}"] = np.ascontiguousarray(w2t[:, :NH8, :]).astype(E4NP)
        wm[f"w2{sfx}"] = np.ascontiguousarray(w2t[:, NH8:, :]).astype(np.float16)
    if np.any(bcs != 0):
        flags["bcs"] = True
        wm["bcs"] = bcs.astype(np.float16).reshape(1, D)
    if np.any(bcg != 0):
        flags["bcg"] = True
        wm["bcg"] = bcg.astype(np.float16).reshape(1, D)
    for nm, key in (("b1s", "seq_b1"), ("b1g", "gr_b1")):
        v = f(key)
        if np.any(v != 0):
            flags[nm] = True
            wm[nm] = np.ascontiguousarray(v.reshape(32, 128).T).astype(np.float32)
    for nm, key in (("b2s", "seq_b2"), ("b2g", "gr_b2")):
        v = f(key)
        if np.any(v != 0):
            flags[nm] = True
            wm[nm] = v.astype(np.float16).reshape(1, D)
    for nm in ("sn1", "sn2", "gn1", "gn2"):
        g = f(f"{nm}_g"); b = f(f"{nm}_b")
        if np.any(g != 1):
            flags[f"{nm}_g"] = True
            wm[f"{nm}_g"] = g.astype(np.float32)
        if np.any(b != 0):
            flags[f"{nm}_b"] = True
            wm[f"{nm}_b"] = b.astype(np.float32)

    seq = np.asarray(f("seq_emb"), dtype=np.float32)
    graph = np.asarray(f("graph_emb"), dtype=np.float32)
    seq16_full = seq.astype(np.float16)
    graph16_full = graph.astype(np.float16)
    in_maps = []
    for i in range(N_CORES):
        m = dict(wm)
        s = seq16_full[i * R:(i + 1) * R]
        g = graph16_full[i * R:(i + 1) * R]
        m["seq16"] = np.ascontiguousarray(s)
        m["graph16"] = np.ascontiguousarray(g)
        m["seqT"] = rhs_tiles(s, 8)
        m["graphT"] = rhs_tiles(g, 8)
        in_maps.append(m)
    return in_maps, flags


def kernel(**inputs):
    in_maps, flags = _host_prep(inputs)
    key = tuple(sorted(flags.items()))
    if key not in _cache:
        _cache[key] = _build(flags)
    nc = _cache[key]
    res = run_bass_kernel_spmd(nc, in_maps, core_ids=list(range(N_CORES)))
    seq_out = np.concatenate([res.results[i]["seq_out"] for i in range(N_CORES)], axis=0)
    graph_out = np.concatenate([res.results[i]["graph_out"] for i in range(N_CORES)], axis=0)
    return (seq_out, graph_out)
